# revision 28
# baseline (speedup 1.0000x reference)
"""Trainium2 Bass kernel for a transformer encoder layer (B=4, S=2048, D=1024, DFF=4096).

Sharding: data-parallel, no collectives. Core c = 2*b + h handles query rows
[b, h*1024:(h+1)*1024]. Each core computes K/V for its full batch (the pair of
cores sharing a batch duplicate that work).

Layout strategy: all attention math runs in "transposed" layouts so no on-device
transposes are needed:
  - X^T via DMA-transpose (host provides fp16/fp8 X),
  - scores computed as scores^T [sk, sq] (k^T stationary, q^T moving),
  - softmax sums over sk (partitions) via a DVE/Pool add-tree + one ones-matmul,
  - intensity supplied pre-transposed by the host,
  - AV^T [d, sq] comes out of the PE directly in the layout the out-proj needs,
  - h1^T (pre-affine z^T) via PE transposes of 128x128 tiles.

Precision: QK/scores and the whole FFN run fp8 DoubleRow (2 rows/cycle); W1/W2
carry power-of-2 prescales (x256 / x64) so their U(-1/32..) ranges land in
e4m3's normal range; the scales come out in the evacuations (f1 is stored at
8x, the FFN2 psum and the h1 residual at 512x -- LayerNorm is scale-invariant).
V/AV/out-proj stay fp16 (their error feeds the residual stream unattenuated).

Scheduling: the PE instruction stream is ordered so every long-latency chain
(softmax denominators, LN1 -> z -> transpose, FFN1 -> f1 evac) is hidden behind
unrelated matmuls; evacuation work is spread across ACT/DVE/Pool so no single
engine gates the PE.
"""

import sys

if "/opt/trn_rl_repo" not in sys.path:
    sys.path.insert(0, "/opt/trn_rl_repo")

import numpy as np

P = 128
B, S, D, DFF = 4, 2048, 1024, 4096
SQ = 1024                 # query rows per core
NK = D // P               # 8  d tiles
NSK = S // P              # 16 sk tiles
NF = DFF // P             # 32 f tiles
NQT = SQ // P             # 8  sq tiles
EPS = 1e-6
SLOPE = 0.01
SCALE = 1.0 / 32.0        # 1/sqrt(D)

_PROG = None


def _build():
    import concourse.mybir as mybir
    import concourse.tile as tile
    from concourse import bacc

    f16 = mybir.dt.float16
    f32 = mybir.dt.float32
    f8 = mybir.dt.float8e4
    Act = mybir.ActivationFunctionType
    Alu = mybir.AluOpType
    DR = mybir.MatmulPerfMode.DoubleRow

    nc = bacc.Bacc("TRN2", debug=False)

    # ---- I/O ----------------------------------------------------------------
    xbT_d = nc.dram_tensor("xbT", [D, S], f16, kind="ExternalInput")
    xbT8_d = nc.dram_tensor("xbT8", [D, S], f8, kind="ExternalInput")
    xh_d = nc.dram_tensor("xh16", [SQ, D], f16, kind="ExternalInput")
    intT_d = nc.dram_tensor("intT", [S, SQ], f16, kind="ExternalInput")
    wq_d = nc.dram_tensor("wq8", [D, D], f8, kind="ExternalInput")
    wk_d = nc.dram_tensor("wk8", [D, D], f8, kind="ExternalInput")
    wv_d = nc.dram_tensor("wv", [D, D], f16, kind="ExternalInput")
    wo_d = nc.dram_tensor("wo", [D, D], f16, kind="ExternalInput")
    # W1 pre-tiled on host to [NF, P(d_in part), NK, P(f)] for contiguous DMA
    w1_d = nc.dram_tensor("w1t4", [NF, P, NK, P], f8, kind="ExternalInput")
    w2_d = nc.dram_tensor("w2", [DFF, D], f8, kind="ExternalInput")
    bq_d = nc.dram_tensor("bq_p", [P, NK], f32, kind="ExternalInput")
    bk_d = nc.dram_tensor("bk_p", [P, NK], f32, kind="ExternalInput")
    bvr_d = nc.dram_tensor("bvr", [P, D], f16, kind="ExternalInput")
    b1p_d = nc.dram_tensor("b1_p", [P, NF], f32, kind="ExternalInput")
    b2r_d = nc.dram_tensor("b2r", [P, D], f16, kind="ExternalInput")
    g1r_d = nc.dram_tensor("g1r", [P, D], f16, kind="ExternalInput")
    out_d = nc.dram_tensor("out", [SQ, D], f16, kind="ExternalOutput")

    def wsl(wd):
        # [D, N] dram -> [P, NK, N] AP (partition-major tiles of contraction dim)
        return wd.rearrange("(o p) n -> p o n", p=P)

    with tile.TileContext(nc) as tc:
        # ---- long-lived pools ----
        cp = tc.alloc_tile_pool(name="consts", bufs=1)
        pp = tc.alloc_tile_pool(name="psum", bufs=6, space="PSUM")
        pps = tc.alloc_tile_pool(name="psrow", bufs=2, space="PSUM")
        sp = tc.alloc_tile_pool(name="stats", bufs=2)

        ident_t = cp.tile([P, P], f16, tag="ident")
        from concourse.masks import make_identity
        make_identity(nc, ident_t)
        ones128_t = cp.tile([P, P], f16, tag="ones128")
        nc.vector.memset(ones128_t, 1.0)
        rinv16_t = cp.tile([1, SQ], f16, tag="rinv16")
        eps_t = cp.tile([P, 1], f32, tag="eps")
        nc.vector.memset(eps_t, EPS)

        # ================= phase A: X^T, k^T, q^T, v =========================
        pv = tc.alloc_tile_pool(name="pV", bufs=1, side="right")
        pkq = tc.alloc_tile_pool(name="pKQ", bufs=1)
        pxt = tc.alloc_tile_pool(name="pXT", bufs=1)
        pw = tc.alloc_tile_pool(name="pW", bufs=2)

        xT_t = pxt.tile([P, NK, S], f16, tag="xT")
        xbT_ap = xbT_d.rearrange("(o p) s -> p o s", p=P)
        xT8_t = pxt.tile([P, NK, S], f8, tag="xT8")
        xbT8_ap = xbT8_d.rearrange("(o p) s -> p o s", p=P)

        kT_t = pkq.tile([P, NK, S], f8, tag="kT")
        qT_t = pkq.tile([P, NK, SQ], f8, tag="qT")
        v_t = pv.tile([P, NSK, D], f16, tag="v")

        # k^T [d_out, sk] = Wk^T @ X^T in fp8 DoubleRow (softmax absorbs the
        # quantization; bias fused into the ACT evacuation)
        wk_t = pw.tile([P, NK, D], f8, tag="wmat8")
        wk_ap = wsl(wk_d)
        # DMAs occupy their issuing engine for the whole transfer in this
        # machine model, so spread: SP takes wk (needed first; ACT is stuck
        # loading its activation table at t=0) then the q/v weights; Pool
        # streams X^T fp8 then fp16; ACT only does the small bias rows.
        nc.sync.dma_start(wk_t, wk_ap)
        for nn in range(S // 512):
            [nc.scalar, nc.sync, nc.sync, nc.sync][nn].dma_start(
                xT8_t[:, :, nn * 512:(nn + 1) * 512],
                xbT8_ap[:, :, nn * 512:(nn + 1) * 512])
        bq_t = cp.tile([P, NK], f32, tag="bq")
        nc.scalar.dma_start(bq_t, bq_d[:, :])
        bk_t = cp.tile([P, NK], f32, tag="bk")
        nc.scalar.dma_start(bk_t, bk_d[:, :])
        bvr_t = cp.tile([P, D], f16, tag="bvr")
        nc.scalar.dma_start(bvr_t, bvr_d[:, :])
        b1p_t = cp.tile([P, NF], f32, tag="b1p")
        nc.scalar.dma_start(b1p_t, b1p_d[:, :])
        for nn in range(S // 512):
            for mo in range(NK):
                ps = pp.tile([P, 512], f32, tag="mm")
                for dj in range(0, NK, 2):
                    nc.tensor.matmul(
                        ps,
                        lhsT=wk_t[:, dj:dj + 2, mo * P:(mo + 1) * P],
                        rhs=xT8_t[:, dj:dj + 2, nn * 512:(nn + 1) * 512],
                        start=(dj == 0),
                        stop=(dj == NK - 2),
                        perf_mode=DR,
                    )
                if mo % 2 == 0:
                    nc.scalar.activation(
                        kT_t[:, mo, nn * 512:(nn + 1) * 512], ps,
                        Act.Identity, bias=bk_t[:, mo:mo + 1], scale=1.0,
                    )
                else:
                    nc.vector.tensor_scalar(
                        kT_t[:, mo, nn * 512:(nn + 1) * 512], ps,
                        bk_t[:, mo:mo + 1], None, Alu.add,
                    )

        # q^T [d_out, sq]  (this core's rows = first SQ columns of X^T)
        wq_t = pw.tile([P, NK, D], f8, tag="wmat8")
        nc.sync.dma_start(xT_t[:, 0:2, :], xbT_ap[:, 0:2, :])
        nc.sync.dma_start(wq_t, wsl(wq_d))
        for oc in range(3):
            nc.gpsimd.dma_start(xT_t[:, 2 + 2 * oc:4 + 2 * oc, :],
                                xbT_ap[:, 2 + 2 * oc:4 + 2 * oc, :])
        for mo in range(NK):
            for nn in range(SQ // 512):
                ps = pp.tile([P, 512], f32, tag="mm")
                for dj in range(0, NK, 2):
                    nc.tensor.matmul(
                        ps,
                        lhsT=wq_t[:, dj:dj + 2, mo * P:(mo + 1) * P],
                        rhs=xT8_t[:, dj:dj + 2, nn * 512:(nn + 1) * 512],
                        start=(dj == 0),
                        stop=(dj == NK - 2),
                        perf_mode=DR,
                    )
                nc.vector.tensor_scalar(
                    qT_t[:, mo, nn * 512:(nn + 1) * 512], ps,
                    bq_t[:, mo:mo + 1], None, Alu.add,
                )

        # v [sk, d] = X @ Wv + bv
        wv_t = pw.tile([P, NK, D], f16, tag="wmat", bufs=1)
        nc.gpsimd.dma_start(wv_t, wsl(wv_d))
        for si in range(NSK):
            for nn in range(D // 512):
                ps = pp.tile([P, 512], f32, tag="mm")
                for di in range(NK):
                    nc.tensor.matmul(
                        ps,
                        lhsT=xT_t[:, di, si * P:(si + 1) * P],
                        rhs=wv_t[:, di, nn * 512:(nn + 1) * 512],
                        start=(di == 0),
                        stop=(di == NK - 1),
                    )
                nc.vector.tensor_tensor(
                    v_t[:, si, nn * 512:(nn + 1) * 512], ps,
                    bvr_t[:, nn * 512:(nn + 1) * 512], Alu.add,
                )

        pw.release()
        pxt.release()

        # ================= phase B: attention ================================
        pe = tc.alloc_tile_pool(name="pE", bufs=1, side="right")
        pint = tc.alloc_tile_pool(name="pInt", bufs=1, side="right")
        expT_t = pe.tile([P, NSK, SQ], f16, tag="expT")
        intT_ap = intT_d.rearrange("(si p) q -> p si q", p=P)

        int_ch = {}

        def int_fetch(nn):
            # bufs=1: chunk 1's DMA implicitly waits for chunk 0's reads
            it = pint.tile([P, NSK, 512], f16, tag="intT")
            nc.sync.dma_start(it, intT_ap[:, :, nn * 512:(nn + 1) * 512])
            int_ch[nn] = it

        int_fetch(0)

        # scores^T [sk, sq] with exp(s/32) fused into the PSUM evacuation
        for nn in range(SQ // 512):
            sl = slice(nn * 512, (nn + 1) * 512)
            for si in range(NSK):
                ps = pp.tile([P, 512], f32, tag="mm")
                for dj in range(0, NK, 2):
                    nc.tensor.matmul(
                        ps,
                        lhsT=kT_t[:, dj:dj + 2, si * P:(si + 1) * P],
                        rhs=qT_t[:, dj:dj + 2, sl],
                        start=(dj == 0),
                        stop=(dj == NK - 2),
                        perf_mode=DR,
                    )
                nc.scalar.activation(
                    expT_t[:, si, sl], ps, Act.Exp, bias=0.0, scale=SCALE,
                )

        # k/q dead after the score matmuls: release before the FFN/out-proj
        # weights go into SBUF. The preload DMAs run on SP (w2/wo/LN rows)
        # and Pool (w1) behind the attention matmuls; ACT keeps doing exps.
        pkq.release()
        pw2 = tc.alloc_tile_pool(name="pW2", bufs=1)
        pw1 = tc.alloc_tile_pool(name="pW1", bufs=1)
        pwo = tc.alloc_tile_pool(name="pWo", bufs=1)
        pln = tc.alloc_tile_pool(name="pLN", bufs=1)
        w2_t = pw2.tile([P, NF, D], f8, tag="w2")
        w2_ap = w2_d.rearrange("(o p) n -> p o n", p=P)
        nc.sync.dma_start(w2_t, w2_ap)
        w1_t = pw1.tile([P, NF, NK, P], f8, tag="w1")
        w1_ap = w1_d.rearrange("f p k q -> p f k q")
        for oc in range(4):
            nc.sync.dma_start(w1_t[:, oc * 8:(oc + 1) * 8, :, :],
                              w1_ap[:, oc * 8:(oc + 1) * 8, :, :])
        wo_t = pwo.tile([P, NK, D], f16, tag="wo")
        nc.sync.dma_start(wo_t, wsl(wo_d))
        g1r_t = pln.tile([P, D], f16, tag="g1r")
        nc.sync.dma_start(g1r_t, g1r_d[:, :])
        b2r_t = pln.tile([P, D], f16, tag="b2r")
        nc.sync.dma_start(b2r_t, b2r_d[:, :])

        eng = [nc.vector, nc.gpsimd]

        onescol_t = cp.tile([P, 1], f16, tag="onescol")
        nc.vector.memset(onescol_t, 1.0)

        def softmax_finish(nn):
            """Denominators on the PE itself (ones-matmul accumulate over the
            16 sk tiles, reciprocal, ones-broadcast back to 128 partitions) so
            the PE never waits on a DVE reduction chain; the normalize ops
            read the broadcast reciprocal straight out of PSUM."""
            sl = slice(nn * 512, (nn + 1) * 512)
            psr = pp.tile([1, 512], f32, tag="mm", name="psr")
            for si in range(NSK):
                nc.tensor.matmul(
                    psr, lhsT=onescol_t, rhs=expT_t[:, si, sl],
                    start=(si == 0), stop=(si == NSK - 1),
                )
            with nc.allow_low_precision(
                reason="softmax denominators; fp16 rel err ~5e-4 is immaterial"
            ):
                nc.vector.reciprocal(rinv16_t[0:1, sl], psr)
            psb = pp.tile([P, 512], f32, tag="mm")
            nc.tensor.matmul(
                psb, lhsT=ones128_t[0:1, :], rhs=rinv16_t[0:1, sl],
                start=True, stop=True,
            )
            it = int_ch.pop(nn)
            for si in range(NSK):
                e = eng[si % 2]
                e.tensor_tensor(expT_t[:, si, sl], expT_t[:, si, sl],
                                psb, Alu.mult)
                e.tensor_tensor(expT_t[:, si, sl], expT_t[:, si, sl],
                                it[:, si, :], Alu.add)

        pav = tc.alloc_tile_pool(name="pAV", bufs=1)
        avT_t = pav.tile([P, NK, SQ], f16, tag="avT")

        def av_chunk(nn):
            """AV^T [d, sq] for one sq chunk: v stationary, attn^T moving."""
            sl = slice(nn * 512, (nn + 1) * 512)
            for mo in range(NK):
                ps = pp.tile([P, 512], f32, tag="mm")
                for si in range(NSK):
                    nc.tensor.matmul(
                        ps,
                        lhsT=v_t[:, si, mo * P:(mo + 1) * P],
                        rhs=expT_t[:, si, sl],
                        start=(si == 0),
                        stop=(si == NSK - 1),
                    )
                nc.scalar.copy(avT_t[:, mo, sl], ps)

        # PE order: psb0 right after the score matmuls (tree0 ran during
        # chunk 1's scores), AV0 while chunk 1 normalizes, then psb1, AV1.
        softmax_finish(0)
        int_fetch(1)
        av_chunk(0)
        softmax_finish(1)
        av_chunk(1)

        pint.release()
        pe.release()
        pv.release()

        # ============ out-proj + residual + LN1 + h1^T (pipelined) ===========
        ph1 = tc.alloc_tile_pool(name="pH1", bufs=1)
        ph1t = tc.alloc_tile_pool(name="pH1T", bufs=1)
        pxh = tc.alloc_tile_pool(name="pXh", bufs=3)

        h1_t = ph1.tile([P, NQT, D], f16, tag="h1")
        h1T_h = [
            ph1t.tile([P, NK, 512], f8, tag="h1T0", name="h1T_0"),
            ph1t.tile([P, NK, 512], f8, tag="h1T1", name="h1T_1"),
        ]

        def outproj(st_):
            """hin[st_] = avT^T @ Wo + (X + bo), fp32."""
            xh = pxh.tile([P, D], f16, tag="xh")
            nc.gpsimd.dma_start(xh, xh_d[st_ * P:(st_ + 1) * P, :])
            hin = pxh.tile([P, D], f32, tag="hin", name=f"hin_{st_}")
            for nn in range(D // 512):
                ps = pp.tile([P, 512], f32, tag="mm")
                for mo in range(NK):
                    nc.tensor.matmul(
                        ps,
                        lhsT=avT_t[:, mo, st_ * P:(st_ + 1) * P],
                        rhs=wo_t[:, mo, nn * 512:(nn + 1) * 512],
                        start=(mo == 0),
                        stop=(mo == NK - 1),
                    )
                nc.vector.tensor_tensor(
                    hin[:, nn * 512:(nn + 1) * 512], ps,
                    xh[:, nn * 512:(nn + 1) * 512], Alu.add,
                )
            return hin

        zs = {}

        def ln1(st_, hin):
            """LayerNorm stats + z (DVE); h1 = 512*(z*g1 + b2 + be1) in f16."""
            st = sp.tile([P, 2, 6], f32, tag="bst")
            nc.vector.bn_stats(st[:, 0, :], hin[:, 0:512])
            nc.vector.bn_stats(st[:, 1, :], hin[:, 512:1024])
            mv = sp.tile([P, 2], f32, tag="mv")
            nc.vector.bn_aggr(mv, st)
            sd = sp.tile([P, 1], f32, tag="sd")
            nc.scalar.activation(sd, mv[:, 1:2], Act.Sqrt, bias=eps_t, scale=1.0)
            rstd = sp.tile([P, 1], f32, tag="rstd")
            nc.vector.reciprocal(rstd, sd)
            nmr = sp.tile([P, 1], f32, tag="nmr")
            nc.vector.tensor_scalar(nmr, mv[:, 0:1], rstd, -1.0,
                                    Alu.mult, Alu.mult)
            z = sp.tile([P, D], f16, tag="z16", bufs=3, name=f"z_{st_}")
            nc.scalar.activation(z, hin, Act.Identity, bias=nmr, scale=rstd)
            zs[st_] = z
            # h1 carries the 512x descale and the (b2+be1) row for FFN2
            ho = h1_t[:, st_, :]
            nc.gpsimd.tensor_tensor(ho, z, g1r_t, Alu.mult)
            nc.gpsimd.tensor_tensor(ho, ho, b2r_t, Alu.add)

        def transpose_z(st_):
            """h1T tiles (f8) for FFN1 via PE transposes of z."""
            z = zs.pop(st_)
            half, stl = divmod(st_, 4)
            for di in range(NK):
                tp = pps.tile([P, P], f16, tag="tp", bufs=2, name="tp")
                nc.tensor.transpose(tp, z[:, di * P:(di + 1) * P], ident_t)
                dst = h1T_h[half][:, di, stl * P:(stl + 1) * P]
                nc.scalar.copy(dst, tp)

        # ================= phase C helpers: FFN ==============================
        pffn = tc.alloc_tile_pool(name="pFFN", bufs=2)
        pout = tc.alloc_tile_pool(name="pOut", bufs=2)
        f1T_h = {}

        def ffn1(half):
            """f1^T [f, sq-half] = 8*leaky(z^T @ W1p + b1p) in fp8 DR.
            Evacuation split: t16 on ACT (even fo) / DVE (odd fo); the leaky
            max as one scalar_tensor_tensor on DVE (even) / Pool (odd)."""
            f1T_t = pffn.tile([P, NF, 512], f8, tag="f1T", name=f"f1T_{half}")
            f1T_h[half] = f1T_t
            for fo in range(NF):
                ps = pp.tile([P, 512], f32, tag="mm")
                for di in range(0, NK, 2):
                    nc.tensor.matmul(
                        ps,
                        lhsT=w1_t[:, fo, di:di + 2, :],
                        rhs=h1T_h[half][:, di:di + 2, :],
                        start=(di == 0),
                        stop=(di == NK - 2),
                        perf_mode=DR,
                    )
                t16 = pout.tile([P, 512], f16, tag="t16", bufs=3)
                if fo % 2 == 0:
                    nc.scalar.activation(
                        t16, ps, Act.Identity, bias=b1p_t[:, fo:fo + 1],
                        scale=0.03125,
                    )
                else:
                    nc.vector.tensor_scalar(
                        t16, ps, 0.03125, b1p_t[:, fo:fo + 1],
                        Alu.mult, Alu.add,
                    )
                eng[fo % 2].scalar_tensor_tensor(
                    f1T_t[:, fo, :], t16, SLOPE, t16, Alu.mult, Alu.max,
                )

        def ffn2(half):
            """hin2 = f1^T^T @ W2 + h1 (all 512-scaled), then LN2 -> out."""
            f1T_t = f1T_h.pop(half)
            for stl in range(4):
                st_ = half * 4 + stl
                hin = pout.tile([P, D], f16, tag="hin2")
                st2 = sp.tile([P, 2, 6], f32, tag="bst")
                for nn in range(D // 512):
                    sl = slice(nn * 512, (nn + 1) * 512)
                    ps = pp.tile([P, 512], f32, tag="mm")
                    for fi in range(0, NF, 2):
                        nc.tensor.matmul(
                            ps,
                            lhsT=f1T_t[:, fi:fi + 2, stl * P:(stl + 1) * P],
                            rhs=w2_t[:, fi:fi + 2, nn * 512:(nn + 1) * 512],
                            start=(fi == 0),
                            stop=(fi == NF - 2),
                            perf_mode=DR,
                        )
                    nc.vector.tensor_tensor(
                        hin[:, sl], ps, h1_t[:, st_, sl], Alu.add,
                    )
                    nc.vector.bn_stats(st2[:, nn, :], hin[:, sl])
                mv = sp.tile([P, 2], f32, tag="mv")
                nc.vector.bn_aggr(mv, st2)
                sd = sp.tile([P, 1], f32, tag="sd")
                nc.scalar.activation(sd, mv[:, 1:2], Act.Sqrt, bias=eps_t,
                                     scale=1.0)
                rstd = sp.tile([P, 1], f32, tag="rstd")
                nc.vector.reciprocal(rstd, sd)
                nmr = sp.tile([P, 1], f32, tag="nmr")
                nc.vector.tensor_scalar(nmr, mv[:, 0:1], rstd, -1.0,
                                        Alu.mult, Alu.mult)
                zo = pout.tile([P, D], f16, tag="zout")
                for ch in range(2):
                    sl = slice(ch * 512, (ch + 1) * 512)
                    # out = normalized(hin); the g2/be2 affine is applied on
                    # the host (it's the last op, nothing downstream on-chip)
                    nc.scalar.activation(zo[:, sl], hin[:, sl], Act.Identity,
                                         bias=nmr, scale=rstd)
                    nc.sync.dma_start(out_d[st_ * P:(st_ + 1) * P, sl],
                                      zo[:, sl])

        # ---- pipelined emission: transposes hide behind the next out-proj;
        # FFN2(half 0) interposes before the last transpose so z7's LN chain
        # has a full matmul block to complete under.
        hins = {}
        hins[0] = outproj(0)
        hins[1] = outproj(1)
        ln1(0, hins.pop(0))
        hins[2] = outproj(2)
        ln1(1, hins.pop(1))
        transpose_z(0)
        hins[3] = outproj(3)
        ln1(2, hins.pop(2))
        transpose_z(1)
        hins[4] = outproj(4)
        ln1(3, hins.pop(3))
        transpose_z(2)
        hins[5] = outproj(5)
        ln1(4, hins.pop(4))
        transpose_z(3)
        ffn1(0)
        hins[6] = outproj(6)
        ln1(5, hins.pop(5))
        transpose_z(4)
        hins[7] = outproj(7)
        ln1(6, hins.pop(6))
        transpose_z(5)
        ffn2(0)
        ln1(7, hins.pop(7))
        transpose_z(6)
        transpose_z(7)
        ffn1(1)
        ffn2(1)

        pout.release()
        pffn.release()
        pxh.release()
        ph1t.release()
        ph1.release()
        pav.release()
        pln.release()
        pwo.release()
        pw1.release()
        pw2.release()
        sp.release()
        pps.release()
        pp.release()
        cp.release()

    nc.finalize()
    return nc


def _host_prep(inputs):
    import ml_dtypes
    f16 = np.float16
    f32 = np.float32
    f8 = ml_dtypes.float8_e4m3fn
    X = np.asarray(inputs["X"], f32)
    I = np.asarray(inputs["intensity"], f32)

    W1 = np.asarray(inputs["W1"], np.float64)
    g1 = np.asarray(inputs["g1"], np.float64)
    be1 = np.asarray(inputs["be1"], np.float64)
    W1p = (W1 * g1[:, None]).astype(np.float32)
    b1p = (np.asarray(inputs["b1"], np.float64) + be1 @ W1).astype(np.float32)
    # fp8 weights with power-of-2 prescales; see the module docstring
    w1t4 = np.ascontiguousarray(
        (256.0 * W1p).astype(f8).reshape(NK, P, NF, P).transpose(2, 1, 0, 3)
    )
    shared = {
        "wq8": np.asarray(inputs["Wq"], np.float32).astype(f8),
        "wk8": np.asarray(inputs["Wk"], np.float32).astype(f8),
        "wv": np.asarray(inputs["Wv"], f16),
        "wo": np.asarray(inputs["Wo"], f16),
        "w1t4": w1t4,
        "w2": (64.0 * np.asarray(inputs["W2"], np.float32)).astype(f8),
        "bq_p": np.ascontiguousarray(np.asarray(inputs["bq"], f32).reshape(NK, P).T),
        "bk_p": np.ascontiguousarray(np.asarray(inputs["bk"], f32).reshape(NK, P).T),
        "bvr": np.ascontiguousarray(
            np.broadcast_to(np.asarray(inputs["bv"], f16)[None, :], (P, D))
        ),
        "b1_p": np.ascontiguousarray((8.0 * b1p).reshape(NF, P).T),
        "b2r": np.ascontiguousarray(np.broadcast_to(
            (512.0 * (np.asarray(inputs["b2"], np.float64)
                      + np.asarray(inputs["be1"], np.float64))
             ).astype(f16)[None, :],
            (P, D))),
        "g1r": np.ascontiguousarray(np.broadcast_to(
            (512.0 * np.asarray(inputs["g1"], np.float64)).astype(f16)[None, :],
            (P, D))),
    }

    in_maps = []
    for c in range(8):
        b, h = divmod(c, 2)
        own = slice(h * SQ, (h + 1) * SQ)
        oth = slice((1 - h) * SQ, (2 - h) * SQ)
        # sk order: own query rows first, then the other half, so q^T is a
        # contiguous slice of X^T. intensity columns follow the same order.
        xb = np.concatenate([X[b, own], X[b, oth]], axis=0)
        Ih = I[b, own]
        intT = np.concatenate([Ih[:, own], Ih[:, oth]], axis=1).T
        m = dict(shared)
        xbT = np.ascontiguousarray(xb.T.astype(f16))
        m["xbT"] = xbT
        m["xbT8"] = xbT.astype(f8)
        m["xh16"] = (X[b, own] + np.asarray(inputs["bo"], f32)[None, :]).astype(f16)
        m["intT"] = np.ascontiguousarray(intT.astype(f16))
        in_maps.append(m)
    return in_maps


def kernel(**inputs) -> np.ndarray:
    global _PROG
    if _PROG is None:
        _PROG = _build()
    from concourse.bass_utils import run_bass_kernel_spmd

    in_maps = _host_prep(inputs)
    res = run_bass_kernel_spmd(_PROG, in_maps, list(range(8)))
    out = np.empty((B, S, D), np.float32)
    for c, r in enumerate(res.results):
        b, h = divmod(c, 2)
        out[b, h * SQ:(h + 1) * SQ] = r["out"]
    g2 = np.asarray(inputs["g2"], np.float32)
    be2 = np.asarray(inputs["be2"], np.float32)
    return out * g2 + be2


# revision 30
# speedup vs baseline: 1.0755x; 1.0755x over previous
"""Trainium2 Bass kernel for a transformer encoder layer (B=4, S=2048, D=1024, DFF=4096).

Sharding: data-parallel, no collectives. Core c = 2*b + h handles query rows
[b, h*1024:(h+1)*1024]. Each core computes K/V for its full batch (the pair of
cores sharing a batch duplicate that work).

Layout strategy: all attention math runs in "transposed" layouts so no on-device
transposes are needed:
  - X^T via DMA-transpose (host provides fp16/fp8 X),
  - scores computed as scores^T [sk, sq] (k^T stationary, q^T moving),
  - softmax sums over sk (partitions) via a DVE/Pool add-tree + one ones-matmul,
  - intensity supplied pre-transposed by the host,
  - AV^T [d, sq] comes out of the PE directly in the layout the out-proj needs,
  - h1^T (pre-affine z^T) via PE transposes of 128x128 tiles.

Precision: QK/scores and the whole FFN run fp8 DoubleRow (2 rows/cycle); W1/W2
carry power-of-2 prescales (x256 / x64) so their U(-1/32..) ranges land in
e4m3's normal range; the scales come out in the evacuations (f1 is stored at
8x, the FFN2 psum and the h1 residual at 512x -- LayerNorm is scale-invariant).
V/AV/out-proj stay fp16 (their error feeds the residual stream unattenuated).

Scheduling: the PE instruction stream is ordered so every long-latency chain
(softmax denominators, LN1 -> z -> transpose, FFN1 -> f1 evac) is hidden behind
unrelated matmuls; evacuation work is spread across ACT/DVE/Pool so no single
engine gates the PE.
"""

import sys

if "/opt/trn_rl_repo" not in sys.path:
    sys.path.insert(0, "/opt/trn_rl_repo")

import numpy as np

P = 128
B, S, D, DFF = 4, 2048, 1024, 4096
SQ = 1024                 # query rows per core
NK = D // P               # 8  d tiles
NSK = S // P              # 16 sk tiles
NF = DFF // P             # 32 f tiles
NQT = SQ // P             # 8  sq tiles
EPS = 1e-6
SLOPE = 0.01
SCALE = 1.0 / 32.0        # 1/sqrt(D)

_PROG = None


def _build():
    import concourse.mybir as mybir
    import concourse.tile as tile
    from concourse import bacc

    f16 = mybir.dt.float16
    f32 = mybir.dt.float32
    f8 = mybir.dt.float8e4
    Act = mybir.ActivationFunctionType
    Alu = mybir.AluOpType
    DR = mybir.MatmulPerfMode.DoubleRow

    nc = bacc.Bacc("TRN2", debug=False)

    # ---- I/O ----------------------------------------------------------------
    xbT_d = nc.dram_tensor("xbT", [D, S], f16, kind="ExternalInput")
    xbT8_d = nc.dram_tensor("xbT8", [D, S], f8, kind="ExternalInput")
    xh_d = nc.dram_tensor("xh16", [SQ, D], f16, kind="ExternalInput")
    intT_d = nc.dram_tensor("intT", [S, SQ], f16, kind="ExternalInput")
    wq_d = nc.dram_tensor("wq8", [D, D], f8, kind="ExternalInput")
    wk_d = nc.dram_tensor("wk8", [D, D], f8, kind="ExternalInput")
    wv_d = nc.dram_tensor("wv", [D, D], f16, kind="ExternalInput")
    wo_d = nc.dram_tensor("wo", [D, D], f16, kind="ExternalInput")
    # W1 pre-tiled on host to [NF, P(d_in part), NK, P(f)] for contiguous DMA
    w1_d = nc.dram_tensor("w1t4", [NF, P, NK, P], f8, kind="ExternalInput")
    w2_d = nc.dram_tensor("w2", [DFF, D], f8, kind="ExternalInput")
    bq_d = nc.dram_tensor("bq_p", [P, NK], f32, kind="ExternalInput")
    bk_d = nc.dram_tensor("bk_p", [P, NK], f32, kind="ExternalInput")
    bvr_d = nc.dram_tensor("bvr", [P, D], f16, kind="ExternalInput")
    b1p_d = nc.dram_tensor("b1_p", [P, NF], f32, kind="ExternalInput")
    b2r_d = nc.dram_tensor("b2r", [P, D], f16, kind="ExternalInput")
    g1r_d = nc.dram_tensor("g1r", [P, D], f16, kind="ExternalInput")
    out_d = nc.dram_tensor("out", [SQ, D], f16, kind="ExternalOutput")

    def wsl(wd):
        # [D, N] dram -> [P, NK, N] AP (partition-major tiles of contraction dim)
        return wd.rearrange("(o p) n -> p o n", p=P)

    with tile.TileContext(nc) as tc:
        # ---- long-lived pools ----
        cp = tc.alloc_tile_pool(name="consts", bufs=1)
        pp = tc.alloc_tile_pool(name="psum", bufs=6, space="PSUM")
        pps = tc.alloc_tile_pool(name="psrow", bufs=2, space="PSUM")
        sp = tc.alloc_tile_pool(name="stats", bufs=2)

        ident_t = cp.tile([P, P], f16, tag="ident")
        from concourse.masks import make_identity
        make_identity(nc, ident_t)
        ones128_t = cp.tile([P, P], f16, tag="ones128")
        nc.vector.memset(ones128_t, 1.0)
        rinvR_t = cp.tile([P, SQ], f16, tag="rinvR")
        eps_t = cp.tile([P, 1], f32, tag="eps")
        nc.vector.memset(eps_t, EPS)

        # ================= phase A: X^T, k^T, q^T, v =========================
        pv = tc.alloc_tile_pool(name="pV", bufs=1, side="right")
        pkq = tc.alloc_tile_pool(name="pKQ", bufs=1)
        pxt = tc.alloc_tile_pool(name="pXT", bufs=1)
        pw = tc.alloc_tile_pool(name="pW", bufs=2)

        xT_t = pxt.tile([P, NK, S], f16, tag="xT")
        xbT_ap = xbT_d.rearrange("(o p) s -> p o s", p=P)
        xT8_t = pxt.tile([P, NK, S], f8, tag="xT8")
        xbT8_ap = xbT8_d.rearrange("(o p) s -> p o s", p=P)

        kT_t = pkq.tile([P, NK, S], f8, tag="kT")
        qT_t = pkq.tile([P, NK, SQ], f8, tag="qT")
        v_t = pv.tile([P, NSK, D], f16, tag="v")

        # k^T [d_out, sk] = Wk^T @ X^T in fp8 DoubleRow (softmax absorbs the
        # quantization; bias fused into the ACT evacuation)
        wk_t = pw.tile([P, NK, D], f8, tag="wmat8")
        wk_ap = wsl(wk_d)
        # DMAs occupy their issuing engine for the whole transfer in this
        # machine model, so spread: SP takes wk (needed first; ACT is stuck
        # loading its activation table at t=0) then the q/v weights; Pool
        # streams X^T fp8 then fp16; ACT only does the small bias rows.
        nc.sync.dma_start(wk_t, wk_ap)
        for nn in range(S // 512):
            [nc.scalar, nc.sync, nc.sync, nc.sync][nn].dma_start(
                xT8_t[:, :, nn * 512:(nn + 1) * 512],
                xbT8_ap[:, :, nn * 512:(nn + 1) * 512])
        bq_t = cp.tile([P, NK], f32, tag="bq")
        nc.scalar.dma_start(bq_t, bq_d[:, :])
        bk_t = cp.tile([P, NK], f32, tag="bk")
        nc.scalar.dma_start(bk_t, bk_d[:, :])
        bvr_t = cp.tile([P, D], f16, tag="bvr")
        nc.scalar.dma_start(bvr_t, bvr_d[:, :])
        b1p_t = cp.tile([P, NF], f32, tag="b1p")
        nc.scalar.dma_start(b1p_t, b1p_d[:, :])
        for nn in range(S // 512):
            for mo in range(NK):
                ps = pp.tile([P, 512], f32, tag="mm")
                for dj in range(0, NK, 2):
                    nc.tensor.matmul(
                        ps,
                        lhsT=wk_t[:, dj:dj + 2, mo * P:(mo + 1) * P],
                        rhs=xT8_t[:, dj:dj + 2, nn * 512:(nn + 1) * 512],
                        start=(dj == 0),
                        stop=(dj == NK - 2),
                        perf_mode=DR,
                    )
                if mo % 2 == 0:
                    nc.scalar.activation(
                        kT_t[:, mo, nn * 512:(nn + 1) * 512], ps,
                        Act.Identity, bias=bk_t[:, mo:mo + 1], scale=1.0,
                    )
                else:
                    nc.vector.tensor_scalar(
                        kT_t[:, mo, nn * 512:(nn + 1) * 512], ps,
                        bk_t[:, mo:mo + 1], None, Alu.add,
                    )

        # q^T [d_out, sq]  (this core's rows = first SQ columns of X^T)
        wq_t = pw.tile([P, NK, D], f8, tag="wmat8")
        nc.sync.dma_start(xT_t[:, 0:2, :], xbT_ap[:, 0:2, :])
        nc.sync.dma_start(wq_t, wsl(wq_d))
        for oc in range(3):
            nc.gpsimd.dma_start(xT_t[:, 2 + 2 * oc:4 + 2 * oc, :],
                                xbT_ap[:, 2 + 2 * oc:4 + 2 * oc, :])
        for mo in range(NK):
            for nn in range(SQ // 512):
                ps = pp.tile([P, 512], f32, tag="mm")
                for dj in range(0, NK, 2):
                    nc.tensor.matmul(
                        ps,
                        lhsT=wq_t[:, dj:dj + 2, mo * P:(mo + 1) * P],
                        rhs=xT8_t[:, dj:dj + 2, nn * 512:(nn + 1) * 512],
                        start=(dj == 0),
                        stop=(dj == NK - 2),
                        perf_mode=DR,
                    )
                nc.vector.tensor_scalar(
                    qT_t[:, mo, nn * 512:(nn + 1) * 512], ps,
                    bq_t[:, mo:mo + 1], None, Alu.add,
                )

        # v [sk, d] = X @ Wv + bv
        wv_t = pw.tile([P, NK, D], f16, tag="wmat", bufs=1)
        nc.gpsimd.dma_start(wv_t, wsl(wv_d))
        for si in range(NSK):
            for nn in range(D // 512):
                ps = pp.tile([P, 512], f32, tag="mm")
                for di in range(NK):
                    nc.tensor.matmul(
                        ps,
                        lhsT=xT_t[:, di, si * P:(si + 1) * P],
                        rhs=wv_t[:, di, nn * 512:(nn + 1) * 512],
                        start=(di == 0),
                        stop=(di == NK - 1),
                    )
                nc.vector.tensor_tensor(
                    v_t[:, si, nn * 512:(nn + 1) * 512], ps,
                    bvr_t[:, nn * 512:(nn + 1) * 512], Alu.add,
                )

        pw.release()
        pxt.release()

        # ================= phase B: attention ================================
        pe = tc.alloc_tile_pool(name="pE", bufs=1, side="right")
        pint = tc.alloc_tile_pool(name="pInt", bufs=1, side="right")
        expT_t = pe.tile([P, NSK, SQ], f16, tag="expT")
        intT_ap = intT_d.rearrange("(si p) q -> p si q", p=P)

        int_ch = {}

        def int_fetch(nn):
            # bufs=1: chunk 1's DMA implicitly waits for chunk 0's reads
            it = pint.tile([P, NSK, 512], f16, tag="intT")
            nc.sync.dma_start(it, intT_ap[:, :, nn * 512:(nn + 1) * 512])
            int_ch[nn] = it

        int_fetch(0)

        # scores^T [sk, sq] with exp(s/32) fused into the PSUM evacuation
        def scores_chunk(nn, mid=None):
            sl = slice(nn * 512, (nn + 1) * 512)
            for si in range(NSK):
                if mid is not None and si == 5:
                    mid()
                ps = pp.tile([P, 512], f32, tag="mm")
                for dj in range(0, NK, 2):
                    nc.tensor.matmul(
                        ps,
                        lhsT=kT_t[:, dj:dj + 2, si * P:(si + 1) * P],
                        rhs=qT_t[:, dj:dj + 2, sl],
                        start=(dj == 0),
                        stop=(dj == NK - 2),
                        perf_mode=DR,
                    )
                nc.scalar.activation(
                    expT_t[:, si, sl], ps, Act.Exp, bias=0.0, scale=SCALE,
                )

        eng = [nc.vector, nc.gpsimd]
        tsums = {}

        def reduce_emit(nn):
            """Denominator partials: DVE X-reduces si 0-3 / 4-7 (start as soon
            as those exps land, during the score matmuls); Pool pairwise-adds
            si 8-15; two DVE adds merge."""
            sl = slice(nn * 512, (nn + 1) * 512)
            tsum = sp.tile([P, 512], f16, tag="dsum")
            tsB = sp.tile([P, 512], f16, tag="dsB")
            tp4 = sp.tile([P, 4, 512], f16, tag="dp4", bufs=1)
            tp2 = sp.tile([P, 2, 512], f16, tag="dp2", bufs=1)
            tpb = sp.tile([P, 512], f16, tag="dpb", bufs=1)
            for j in range(4):
                nc.gpsimd.tensor_tensor(
                    tp4[:, j, :], expT_t[:, 8 + 2 * j, sl],
                    expT_t[:, 9 + 2 * j, sl], Alu.add,
                )
            for j in range(2):
                nc.gpsimd.tensor_tensor(
                    tp2[:, j, :], tp4[:, 2 * j, :], tp4[:, 2 * j + 1, :],
                    Alu.add,
                )
            nc.gpsimd.tensor_tensor(tpb, tp2[:, 0, :], tp2[:, 1, :], Alu.add)
            with nc.allow_low_precision(
                reason="softmax denominators; fp16 rel err ~5e-4 is immaterial"
            ):
                nc.vector.tensor_reduce(
                    tsum, expT_t[:, 0:4, sl].rearrange("p a b -> p b a"),
                    mybir.AxisListType.X, Alu.add,
                )
                nc.vector.tensor_reduce(
                    tsB, expT_t[:, 4:8, sl].rearrange("p a b -> p b a"),
                    mybir.AxisListType.X, Alu.add,
                )
            nc.vector.tensor_tensor(tsum, tsum, tsB, Alu.add)
            nc.vector.tensor_tensor(tsum, tsum, tpb, Alu.add)
            tsums[nn] = tsum

        def softmax_finish(nn):
            """Broadcast the partition-sum via a ones-matmul (PE), reciprocal,
            then normalize + add intensity in place; si<10 on DVE, rest on
            Pool, matching AV's in-order consumption."""
            sl = slice(nn * 512, (nn + 1) * 512)
            psb = pp.tile([P, 512], f32, tag="mm")
            nc.tensor.matmul(
                psb, lhsT=ones128_t, rhs=tsums.pop(nn), start=True, stop=True,
            )
            with nc.allow_low_precision(
                reason="softmax denominators; fp16 rel err ~5e-4 is immaterial"
            ):
                nc.vector.reciprocal(rinvR_t[:, sl], psb)
            it = int_ch.pop(nn)
            for si in range(NSK):
                e = eng[0 if si < 10 else 1]
                e.tensor_tensor(expT_t[:, si, sl], expT_t[:, si, sl],
                                rinvR_t[:, sl], Alu.mult)
                e.tensor_tensor(expT_t[:, si, sl], expT_t[:, si, sl],
                                it[:, si, :], Alu.add)

        scores_chunk(0)
        reduce_emit(0)
        scores_chunk(1, mid=lambda: softmax_finish(0))

        # k/q dead after the score matmuls: release before the FFN/out-proj
        # weights go into SBUF. The preload DMAs all run on SP's hardware DGE
        # behind the attention matmuls; ACT keeps doing exps.
        pkq.release()
        pw2 = tc.alloc_tile_pool(name="pW2", bufs=1)
        pw1 = tc.alloc_tile_pool(name="pW1", bufs=1)
        pwo = tc.alloc_tile_pool(name="pWo", bufs=1)
        pln = tc.alloc_tile_pool(name="pLN", bufs=1)
        w2_t = pw2.tile([P, NF, D], f8, tag="w2")
        w2_ap = w2_d.rearrange("(o p) n -> p o n", p=P)
        nc.sync.dma_start(w2_t, w2_ap)
        w1_t = pw1.tile([P, NF, NK, P], f8, tag="w1")
        w1_ap = w1_d.rearrange("f p k q -> p f k q")
        for oc in range(4):
            nc.sync.dma_start(w1_t[:, oc * 8:(oc + 1) * 8, :, :],
                              w1_ap[:, oc * 8:(oc + 1) * 8, :, :])
        wo_t = pwo.tile([P, NK, D], f16, tag="wo")
        nc.sync.dma_start(wo_t, wsl(wo_d))
        g1r_t = pln.tile([P, D], f16, tag="g1r")
        nc.sync.dma_start(g1r_t, g1r_d[:, :])
        b2r_t = pln.tile([P, D], f16, tag="b2r")
        nc.sync.dma_start(b2r_t, b2r_d[:, :])

        pav = tc.alloc_tile_pool(name="pAV", bufs=1)
        avT_t = pav.tile([P, NK, SQ], f16, tag="avT")

        def av_chunk(nn, mid=None):
            """AV^T [d, sq] for one sq chunk: v stationary, attn^T moving."""
            sl = slice(nn * 512, (nn + 1) * 512)
            for mo in range(NK):
                if mid is not None and mo == 3:
                    mid()
                ps = pp.tile([P, 512], f32, tag="mm")
                for si in range(NSK):
                    nc.tensor.matmul(
                        ps,
                        lhsT=v_t[:, si, mo * P:(mo + 1) * P],
                        rhs=expT_t[:, si, sl],
                        start=(si == 0),
                        stop=(si == NSK - 1),
                    )
                nc.scalar.copy(avT_t[:, mo, sl], ps)

        int_fetch(1)
        reduce_emit(1)
        av_chunk(0, mid=lambda: softmax_finish(1))
        av_chunk(1)

        pint.release()
        pe.release()
        pv.release()

        # ============ out-proj + residual + LN1 + h1^T (pipelined) ===========
        ph1 = tc.alloc_tile_pool(name="pH1", bufs=1)
        ph1t = tc.alloc_tile_pool(name="pH1T", bufs=1)
        pxh = tc.alloc_tile_pool(name="pXh", bufs=3)

        h1_t = ph1.tile([P, NQT, D], f16, tag="h1")
        h1T_h = [
            ph1t.tile([P, NK, 512], f8, tag="h1T0", name="h1T_0"),
            ph1t.tile([P, NK, 512], f8, tag="h1T1", name="h1T_1"),
        ]

        def outproj(st_):
            """hin[st_] = avT^T @ Wo + (X + bo), fp32."""
            xh = pxh.tile([P, D], f16, tag="xh")
            nc.gpsimd.dma_start(xh, xh_d[st_ * P:(st_ + 1) * P, :])
            hin = pxh.tile([P, D], f32, tag="hin", name=f"hin_{st_}")
            for nn in range(D // 512):
                ps = pp.tile([P, 512], f32, tag="mm")
                for mo in range(NK):
                    nc.tensor.matmul(
                        ps,
                        lhsT=avT_t[:, mo, st_ * P:(st_ + 1) * P],
                        rhs=wo_t[:, mo, nn * 512:(nn + 1) * 512],
                        start=(mo == 0),
                        stop=(mo == NK - 1),
                    )
                nc.vector.tensor_tensor(
                    hin[:, nn * 512:(nn + 1) * 512], ps,
                    xh[:, nn * 512:(nn + 1) * 512], Alu.add,
                )
            return hin

        zs = {}

        def ln1(st_, hin):
            """LayerNorm stats + z (DVE); h1 = 512*(z*g1 + b2 + be1) in f16."""
            st = sp.tile([P, 2, 6], f32, tag="bst")
            nc.vector.bn_stats(st[:, 0, :], hin[:, 0:512])
            nc.vector.bn_stats(st[:, 1, :], hin[:, 512:1024])
            mv = sp.tile([P, 2], f32, tag="mv")
            nc.vector.bn_aggr(mv, st)
            sd = sp.tile([P, 1], f32, tag="sd")
            nc.scalar.activation(sd, mv[:, 1:2], Act.Sqrt, bias=eps_t, scale=1.0)
            rstd = sp.tile([P, 1], f32, tag="rstd")
            nc.vector.reciprocal(rstd, sd)
            nmr = sp.tile([P, 1], f32, tag="nmr")
            nc.vector.tensor_scalar(nmr, mv[:, 0:1], rstd, -1.0,
                                    Alu.mult, Alu.mult)
            z = sp.tile([P, D], f16, tag="z16", bufs=3, name=f"z_{st_}")
            nc.scalar.activation(z, hin, Act.Identity, bias=nmr, scale=rstd)
            zs[st_] = z
            # h1 carries the 512x descale and the (b2+be1) row for FFN2
            ho = h1_t[:, st_, :]
            nc.gpsimd.tensor_tensor(ho, z, g1r_t, Alu.mult)
            nc.gpsimd.tensor_tensor(ho, ho, b2r_t, Alu.add)

        def transpose_z(st_):
            """h1T tiles (f8) for FFN1 via PE transposes of z."""
            z = zs.pop(st_)
            half, stl = divmod(st_, 4)
            for di in range(NK):
                tp = pps.tile([P, P], f16, tag="tp", bufs=2, name="tp")
                nc.tensor.transpose(tp, z[:, di * P:(di + 1) * P], ident_t)
                dst = h1T_h[half][:, di, stl * P:(stl + 1) * P]
                nc.scalar.copy(dst, tp)

        # ================= phase C helpers: FFN ==============================
        pffn = tc.alloc_tile_pool(name="pFFN", bufs=2)
        pout = tc.alloc_tile_pool(name="pOut", bufs=2)
        f1T_h = {}

        def ffn1(half):
            """f1^T [f, sq-half] = 8*leaky(z^T @ W1p + b1p) in fp8 DR.
            Evacuation split: t16 on ACT (even fo) / DVE (odd fo); the leaky
            max as one scalar_tensor_tensor on DVE (even) / Pool (odd)."""
            f1T_t = pffn.tile([P, NF, 512], f8, tag="f1T", name=f"f1T_{half}")
            f1T_h[half] = f1T_t
            for fo in range(NF):
                ps = pp.tile([P, 512], f32, tag="mm")
                for di in range(0, NK, 2):
                    nc.tensor.matmul(
                        ps,
                        lhsT=w1_t[:, fo, di:di + 2, :],
                        rhs=h1T_h[half][:, di:di + 2, :],
                        start=(di == 0),
                        stop=(di == NK - 2),
                        perf_mode=DR,
                    )
                t16 = pout.tile([P, 512], f16, tag="t16", bufs=3)
                if fo % 3 != 2:
                    nc.scalar.activation(
                        t16, ps, Act.Identity, bias=b1p_t[:, fo:fo + 1],
                        scale=0.03125,
                    )
                else:
                    nc.vector.tensor_scalar(
                        t16, ps, 0.03125, b1p_t[:, fo:fo + 1],
                        Alu.mult, Alu.add,
                    )
                eng[1 if fo % 3 != 1 else 0].scalar_tensor_tensor(
                    f1T_t[:, fo, :], t16, SLOPE, t16, Alu.mult, Alu.max,
                )

        def ffn2(half):
            """hin2 = f1^T^T @ W2 + h1 (all 512-scaled), then LN2 -> out."""
            f1T_t = f1T_h.pop(half)
            for stl in range(4):
                st_ = half * 4 + stl
                hin = pout.tile([P, D], f16, tag="hin2")
                st2 = sp.tile([P, 2, 6], f32, tag="bst")
                for nn in range(D // 512):
                    sl = slice(nn * 512, (nn + 1) * 512)
                    ps = pp.tile([P, 512], f32, tag="mm")
                    for fi in range(0, NF, 2):
                        nc.tensor.matmul(
                            ps,
                            lhsT=f1T_t[:, fi:fi + 2, stl * P:(stl + 1) * P],
                            rhs=w2_t[:, fi:fi + 2, nn * 512:(nn + 1) * 512],
                            start=(fi == 0),
                            stop=(fi == NF - 2),
                            perf_mode=DR,
                        )
                    nc.vector.tensor_tensor(
                        hin[:, sl], ps, h1_t[:, st_, sl], Alu.add,
                    )
                    nc.vector.bn_stats(st2[:, nn, :], hin[:, sl])
                mv = sp.tile([P, 2], f32, tag="mv")
                nc.vector.bn_aggr(mv, st2)
                sd = sp.tile([P, 1], f32, tag="sd")
                nc.scalar.activation(sd, mv[:, 1:2], Act.Sqrt, bias=eps_t,
                                     scale=1.0)
                rstd = sp.tile([P, 1], f32, tag="rstd")
                nc.vector.reciprocal(rstd, sd)
                nmr = sp.tile([P, 1], f32, tag="nmr")
                nc.vector.tensor_scalar(nmr, mv[:, 0:1], rstd, -1.0,
                                        Alu.mult, Alu.mult)
                zo = pout.tile([P, D], f16, tag="zout")
                for ch in range(2):
                    sl = slice(ch * 512, (ch + 1) * 512)
                    # out = normalized(hin); the g2/be2 affine is applied on
                    # the host (it's the last op, nothing downstream on-chip)
                    nc.scalar.activation(zo[:, sl], hin[:, sl], Act.Identity,
                                         bias=nmr, scale=rstd)
                    nc.sync.dma_start(out_d[st_ * P:(st_ + 1) * P, sl],
                                      zo[:, sl])

        # ---- pipelined emission: transposes hide behind the next out-proj;
        # FFN2(half 0) interposes before the last transpose so z7's LN chain
        # has a full matmul block to complete under.
        hins = {}
        hins[0] = outproj(0)
        hins[1] = outproj(1)
        ln1(0, hins.pop(0))
        hins[2] = outproj(2)
        ln1(1, hins.pop(1))
        transpose_z(0)
        hins[3] = outproj(3)
        ln1(2, hins.pop(2))
        transpose_z(1)
        hins[4] = outproj(4)
        ln1(3, hins.pop(3))
        transpose_z(2)
        hins[5] = outproj(5)
        ln1(4, hins.pop(4))
        transpose_z(3)
        ffn1(0)
        hins[6] = outproj(6)
        ln1(5, hins.pop(5))
        transpose_z(4)
        hins[7] = outproj(7)
        ln1(6, hins.pop(6))
        transpose_z(5)
        ln1(7, hins.pop(7))
        ffn2(0)
        transpose_z(6)
        transpose_z(7)
        ffn1(1)
        ffn2(1)

        pout.release()
        pffn.release()
        pxh.release()
        ph1t.release()
        ph1.release()
        pav.release()
        pln.release()
        pwo.release()
        pw1.release()
        pw2.release()
        sp.release()
        pps.release()
        pp.release()
        cp.release()

    nc.finalize()
    return nc


def _host_prep(inputs):
    import ml_dtypes
    f16 = np.float16
    f32 = np.float32
    f8 = ml_dtypes.float8_e4m3fn
    X = np.asarray(inputs["X"], f32)
    I = np.asarray(inputs["intensity"], f32)

    W1 = np.asarray(inputs["W1"], np.float64)
    g1 = np.asarray(inputs["g1"], np.float64)
    be1 = np.asarray(inputs["be1"], np.float64)
    W1p = (W1 * g1[:, None]).astype(np.float32)
    b1p = (np.asarray(inputs["b1"], np.float64) + be1 @ W1).astype(np.float32)
    # fp8 weights with power-of-2 prescales; see the module docstring
    w1t4 = np.ascontiguousarray(
        (256.0 * W1p).astype(f8).reshape(NK, P, NF, P).transpose(2, 1, 0, 3)
    )
    shared = {
        "wq8": np.asarray(inputs["Wq"], np.float32).astype(f8),
        "wk8": np.asarray(inputs["Wk"], np.float32).astype(f8),
        "wv": np.asarray(inputs["Wv"], f16),
        "wo": np.asarray(inputs["Wo"], f16),
        "w1t4": w1t4,
        "w2": (64.0 * np.asarray(inputs["W2"], np.float32)).astype(f8),
        "bq_p": np.ascontiguousarray(np.asarray(inputs["bq"], f32).reshape(NK, P).T),
        "bk_p": np.ascontiguousarray(np.asarray(inputs["bk"], f32).reshape(NK, P).T),
        "bvr": np.ascontiguousarray(
            np.broadcast_to(np.asarray(inputs["bv"], f16)[None, :], (P, D))
        ),
        "b1_p": np.ascontiguousarray((8.0 * b1p).reshape(NF, P).T),
        "b2r": np.ascontiguousarray(np.broadcast_to(
            (512.0 * (np.asarray(inputs["b2"], np.float64)
                      + np.asarray(inputs["be1"], np.float64))
             ).astype(f16)[None, :],
            (P, D))),
        "g1r": np.ascontiguousarray(np.broadcast_to(
            (512.0 * np.asarray(inputs["g1"], np.float64)).astype(f16)[None, :],
            (P, D))),
    }

    in_maps = []
    for c in range(8):
        b, h = divmod(c, 2)
        own = slice(h * SQ, (h + 1) * SQ)
        oth = slice((1 - h) * SQ, (2 - h) * SQ)
        # sk order: own query rows first, then the other half, so q^T is a
        # contiguous slice of X^T. intensity columns follow the same order.
        xb = np.concatenate([X[b, own], X[b, oth]], axis=0)
        Ih = I[b, own]
        intT = np.concatenate([Ih[:, own], Ih[:, oth]], axis=1).T
        m = dict(shared)
        xbT = np.ascontiguousarray(xb.T.astype(f16))
        m["xbT"] = xbT
        m["xbT8"] = xbT.astype(f8)
        m["xh16"] = (X[b, own] + np.asarray(inputs["bo"], f32)[None, :]).astype(f16)
        m["intT"] = np.ascontiguousarray(intT.astype(f16))
        in_maps.append(m)
    return in_maps


def kernel(**inputs) -> np.ndarray:
    global _PROG
    if _PROG is None:
        _PROG = _build()
    from concourse.bass_utils import run_bass_kernel_spmd

    in_maps = _host_prep(inputs)
    res = run_bass_kernel_spmd(_PROG, in_maps, list(range(8)))
    out = np.empty((B, S, D), np.float32)
    for c, r in enumerate(res.results):
        b, h = divmod(c, 2)
        out[b, h * SQ:(h + 1) * SQ] = r["out"]
    g2 = np.asarray(inputs["g2"], np.float32)
    be2 = np.asarray(inputs["be2"], np.float32)
    return out * g2 + be2


# revision 31
# speedup vs baseline: 1.0934x; 1.0167x over previous
"""Trainium2 Bass kernel for a transformer encoder layer (B=4, S=2048, D=1024, DFF=4096).

Sharding: data-parallel, no collectives. Core c = 2*b + h handles query rows
[b, h*1024:(h+1)*1024]. Each core computes K/V for its full batch (the pair of
cores sharing a batch duplicate that work).

Layout strategy: all attention math runs in "transposed" layouts so no on-device
transposes are needed:
  - X^T via DMA-transpose (host provides fp16/fp8 X),
  - scores computed as scores^T [sk, sq] (k^T stationary, q^T moving),
  - softmax sums over sk (partitions) via a DVE/Pool add-tree + one ones-matmul,
  - intensity supplied pre-transposed by the host,
  - AV^T [d, sq] comes out of the PE directly in the layout the out-proj needs,
  - h1^T (pre-affine z^T) via PE transposes of 128x128 tiles.

Precision: QK/scores and the whole FFN run fp8 DoubleRow (2 rows/cycle); W1/W2
carry power-of-2 prescales (x256 / x64) so their U(-1/32..) ranges land in
e4m3's normal range; the scales come out in the evacuations (f1 is stored at
8x, the FFN2 psum and the h1 residual at 512x -- LayerNorm is scale-invariant).
V/AV/out-proj stay fp16 (their error feeds the residual stream unattenuated).

Scheduling: the PE instruction stream is ordered so every long-latency chain
(softmax denominators, LN1 -> z -> transpose, FFN1 -> f1 evac) is hidden behind
unrelated matmuls; evacuation work is spread across ACT/DVE/Pool so no single
engine gates the PE.
"""

import sys

if "/opt/trn_rl_repo" not in sys.path:
    sys.path.insert(0, "/opt/trn_rl_repo")

import numpy as np

P = 128
B, S, D, DFF = 4, 2048, 1024, 4096
SQ = 1024                 # query rows per core
NK = D // P               # 8  d tiles
NSK = S // P              # 16 sk tiles
NF = DFF // P             # 32 f tiles
NQT = SQ // P             # 8  sq tiles
EPS = 1e-6
SLOPE = 0.01
SCALE = 1.0 / 32.0        # 1/sqrt(D)

_PROG = None


def _build():
    import concourse.mybir as mybir
    import concourse.tile as tile
    from concourse import bacc

    f16 = mybir.dt.float16
    f32 = mybir.dt.float32
    f8 = mybir.dt.float8e4
    Act = mybir.ActivationFunctionType
    Alu = mybir.AluOpType
    DR = mybir.MatmulPerfMode.DoubleRow

    nc = bacc.Bacc("TRN2", debug=False)

    # ---- I/O ----------------------------------------------------------------
    xbT_d = nc.dram_tensor("xbT", [D, S], f16, kind="ExternalInput")
    xbT8_d = nc.dram_tensor("xbT8", [D, S], f8, kind="ExternalInput")
    xh_d = nc.dram_tensor("xh16", [SQ, D], f16, kind="ExternalInput")
    intT_d = nc.dram_tensor("intT", [S, SQ], f16, kind="ExternalInput")
    wq_d = nc.dram_tensor("wq8", [D, D], f8, kind="ExternalInput")
    wk_d = nc.dram_tensor("wk8", [D, D], f8, kind="ExternalInput")
    wv_d = nc.dram_tensor("wv", [D, D], f16, kind="ExternalInput")
    wo_d = nc.dram_tensor("wo", [D, D], f16, kind="ExternalInput")
    # W1 pre-tiled on host to [NF, P(d_in part), NK, P(f)] for contiguous DMA
    w1_d = nc.dram_tensor("w1t4", [NF, P, NK, P], f8, kind="ExternalInput")
    w2_d = nc.dram_tensor("w2", [DFF, D], f8, kind="ExternalInput")
    bq_d = nc.dram_tensor("bq_p", [P, NK], f32, kind="ExternalInput")
    bk_d = nc.dram_tensor("bk_p", [P, NK], f32, kind="ExternalInput")
    bvr_d = nc.dram_tensor("bvr", [P, D], f16, kind="ExternalInput")
    b1p_d = nc.dram_tensor("b1_p", [P, NF], f32, kind="ExternalInput")
    b2r_d = nc.dram_tensor("b2r", [P, D], f16, kind="ExternalInput")
    g1r_d = nc.dram_tensor("g1r", [P, D], f16, kind="ExternalInput")
    out_d = nc.dram_tensor("out", [SQ, D], f16, kind="ExternalOutput")

    def wsl(wd):
        # [D, N] dram -> [P, NK, N] AP (partition-major tiles of contraction dim)
        return wd.rearrange("(o p) n -> p o n", p=P)

    with tile.TileContext(nc) as tc:
        # ---- long-lived pools ----
        cp = tc.alloc_tile_pool(name="consts", bufs=1)
        pp = tc.alloc_tile_pool(name="psum", bufs=6, space="PSUM")
        pps = tc.alloc_tile_pool(name="psrow", bufs=2, space="PSUM")
        sp = tc.alloc_tile_pool(name="stats", bufs=2)

        ident_t = cp.tile([P, P], f16, tag="ident")
        from concourse.masks import make_identity
        make_identity(nc, ident_t)
        ones128_t = cp.tile([P, P], f16, tag="ones128")
        nc.vector.memset(ones128_t, 1.0)
        rinvR_t = cp.tile([P, SQ], f16, tag="rinvR")
        eps_t = cp.tile([P, 1], f32, tag="eps")
        nc.vector.memset(eps_t, EPS)

        # ================= phase A: X^T, k^T, q^T, v =========================
        pv = tc.alloc_tile_pool(name="pV", bufs=1, side="right")
        pkq = tc.alloc_tile_pool(name="pKQ", bufs=1)
        pxt = tc.alloc_tile_pool(name="pXT", bufs=1)
        pw = tc.alloc_tile_pool(name="pW", bufs=2)

        xT_t = pxt.tile([P, NK, S], f16, tag="xT")
        xbT_ap = xbT_d.rearrange("(o p) s -> p o s", p=P)
        xT8_t = pxt.tile([P, NK, S], f8, tag="xT8")
        xbT8_ap = xbT8_d.rearrange("(o p) s -> p o s", p=P)

        kT_t = pkq.tile([P, NK, S], f8, tag="kT")
        qT_t = pkq.tile([P, NK, SQ], f8, tag="qT")
        v_t = pv.tile([P, NSK, D], f16, tag="v")

        # k^T [d_out, sk] = Wk^T @ X^T in fp8 DoubleRow (softmax absorbs the
        # quantization; bias fused into the ACT evacuation)
        wk_t = pw.tile([P, NK, D], f8, tag="wmat8")
        wk_ap = wsl(wk_d)
        # DMAs occupy their issuing engine for the whole transfer in this
        # machine model, so spread: SP takes wk (needed first; ACT is stuck
        # loading its activation table at t=0) then the q/v weights; Pool
        # streams X^T fp8 then fp16; ACT only does the small bias rows.
        nc.sync.dma_start(wk_t[:, 0:4, :], wk_ap[:, 0:4, :])
        nc.gpsimd.dma_start(wk_t[:, 4:8, :], wk_ap[:, 4:8, :])
        for nn in range(S // 512):
            [nc.scalar, nc.sync, nc.gpsimd, nc.sync][nn].dma_start(
                xT8_t[:, :, nn * 512:(nn + 1) * 512],
                xbT8_ap[:, :, nn * 512:(nn + 1) * 512])
        bq_t = cp.tile([P, NK], f32, tag="bq")
        nc.scalar.dma_start(bq_t, bq_d[:, :])
        bk_t = cp.tile([P, NK], f32, tag="bk")
        nc.scalar.dma_start(bk_t, bk_d[:, :])
        bvr_t = cp.tile([P, D], f16, tag="bvr")
        nc.scalar.dma_start(bvr_t, bvr_d[:, :])
        b1p_t = cp.tile([P, NF], f32, tag="b1p")
        nc.scalar.dma_start(b1p_t, b1p_d[:, :])
        for nn in range(S // 512):
            for mo in range(NK):
                ps = pp.tile([P, 512], f32, tag="mm")
                for dj in range(0, NK, 2):
                    nc.tensor.matmul(
                        ps,
                        lhsT=wk_t[:, dj:dj + 2, mo * P:(mo + 1) * P],
                        rhs=xT8_t[:, dj:dj + 2, nn * 512:(nn + 1) * 512],
                        start=(dj == 0),
                        stop=(dj == NK - 2),
                        perf_mode=DR,
                    )
                if mo % 2 == 0:
                    nc.scalar.activation(
                        kT_t[:, mo, nn * 512:(nn + 1) * 512], ps,
                        Act.Identity, bias=bk_t[:, mo:mo + 1], scale=1.0,
                    )
                else:
                    nc.vector.tensor_scalar(
                        kT_t[:, mo, nn * 512:(nn + 1) * 512], ps,
                        bk_t[:, mo:mo + 1], None, Alu.add,
                    )

        # q^T [d_out, sq]  (this core's rows = first SQ columns of X^T)
        wq_t = pw.tile([P, NK, D], f8, tag="wmat8")
        nc.sync.dma_start(xT_t[:, 0:2, :], xbT_ap[:, 0:2, :])
        nc.sync.dma_start(wq_t, wsl(wq_d))
        for oc in range(3):
            nc.gpsimd.dma_start(xT_t[:, 2 + 2 * oc:4 + 2 * oc, :],
                                xbT_ap[:, 2 + 2 * oc:4 + 2 * oc, :])
        for mo in range(NK):
            for nn in range(SQ // 512):
                ps = pp.tile([P, 512], f32, tag="mm")
                for dj in range(0, NK, 2):
                    nc.tensor.matmul(
                        ps,
                        lhsT=wq_t[:, dj:dj + 2, mo * P:(mo + 1) * P],
                        rhs=xT8_t[:, dj:dj + 2, nn * 512:(nn + 1) * 512],
                        start=(dj == 0),
                        stop=(dj == NK - 2),
                        perf_mode=DR,
                    )
                nc.vector.tensor_scalar(
                    qT_t[:, mo, nn * 512:(nn + 1) * 512], ps,
                    bq_t[:, mo:mo + 1], None, Alu.add,
                )

        # v [sk, d] = X @ Wv + bv
        wv_t = pw.tile([P, NK, D], f16, tag="wmat", bufs=1)
        nc.gpsimd.dma_start(wv_t, wsl(wv_d))
        for si in range(NSK):
            for nn in range(D // 512):
                ps = pp.tile([P, 512], f32, tag="mm")
                for di in range(NK):
                    nc.tensor.matmul(
                        ps,
                        lhsT=xT_t[:, di, si * P:(si + 1) * P],
                        rhs=wv_t[:, di, nn * 512:(nn + 1) * 512],
                        start=(di == 0),
                        stop=(di == NK - 1),
                    )
                nc.vector.tensor_tensor(
                    v_t[:, si, nn * 512:(nn + 1) * 512], ps,
                    bvr_t[:, nn * 512:(nn + 1) * 512], Alu.add,
                )

        pw.release()
        pxt.release()

        # ================= phase B: attention ================================
        pe = tc.alloc_tile_pool(name="pE", bufs=1, side="right")
        pint = tc.alloc_tile_pool(name="pInt", bufs=1, side="right")
        expT_t = pe.tile([P, NSK, SQ], f16, tag="expT")
        intT_ap = intT_d.rearrange("(si p) q -> p si q", p=P)

        int_ch = {}

        def int_fetch(nn):
            # bufs=1: chunk 1's DMA implicitly waits for chunk 0's reads
            it = pint.tile([P, NSK, 512], f16, tag="intT")
            nc.sync.dma_start(it, intT_ap[:, :, nn * 512:(nn + 1) * 512])
            int_ch[nn] = it

        int_fetch(0)

        # scores^T [sk, sq] with exp(s/32) fused into the PSUM evacuation
        def scores_chunk(nn, mid=None):
            sl = slice(nn * 512, (nn + 1) * 512)
            for si in range(NSK):
                if mid is not None and si == 5:
                    mid()
                ps = pp.tile([P, 512], f32, tag="mm")
                for dj in range(0, NK, 2):
                    nc.tensor.matmul(
                        ps,
                        lhsT=kT_t[:, dj:dj + 2, si * P:(si + 1) * P],
                        rhs=qT_t[:, dj:dj + 2, sl],
                        start=(dj == 0),
                        stop=(dj == NK - 2),
                        perf_mode=DR,
                    )
                nc.scalar.activation(
                    expT_t[:, si, sl], ps, Act.Exp, bias=0.0, scale=SCALE,
                )

        eng = [nc.vector, nc.gpsimd]
        tsums = {}

        def reduce_emit(nn):
            """Denominator partials: DVE X-reduces si 0-3 / 4-7 (start as soon
            as those exps land, during the score matmuls); Pool pairwise-adds
            si 8-15; two DVE adds merge."""
            sl = slice(nn * 512, (nn + 1) * 512)
            tsum = sp.tile([P, 512], f16, tag="dsum")
            tsB = sp.tile([P, 512], f16, tag="dsB")
            tp4 = sp.tile([P, 4, 512], f16, tag="dp4", bufs=1)
            tp2 = sp.tile([P, 2, 512], f16, tag="dp2", bufs=1)
            tpb = sp.tile([P, 512], f16, tag="dpb", bufs=1)
            for j in range(4):
                nc.gpsimd.tensor_tensor(
                    tp4[:, j, :], expT_t[:, 8 + 2 * j, sl],
                    expT_t[:, 9 + 2 * j, sl], Alu.add,
                )
            for j in range(2):
                nc.gpsimd.tensor_tensor(
                    tp2[:, j, :], tp4[:, 2 * j, :], tp4[:, 2 * j + 1, :],
                    Alu.add,
                )
            nc.gpsimd.tensor_tensor(tpb, tp2[:, 0, :], tp2[:, 1, :], Alu.add)
            with nc.allow_low_precision(
                reason="softmax denominators; fp16 rel err ~5e-4 is immaterial"
            ):
                nc.vector.tensor_reduce(
                    tsum, expT_t[:, 0:4, sl].rearrange("p a b -> p b a"),
                    mybir.AxisListType.X, Alu.add,
                )
                nc.vector.tensor_reduce(
                    tsB, expT_t[:, 4:8, sl].rearrange("p a b -> p b a"),
                    mybir.AxisListType.X, Alu.add,
                )
            nc.gpsimd.tensor_tensor(tsum, tsum, tsB, Alu.add)
            nc.gpsimd.tensor_tensor(tsum, tsum, tpb, Alu.add)
            tsums[nn] = tsum

        def softmax_finish(nn):
            """Broadcast the partition-sum via a ones-matmul (PE), reciprocal,
            then normalize + add intensity in place; si<10 on DVE, rest on
            Pool, matching AV's in-order consumption."""
            sl = slice(nn * 512, (nn + 1) * 512)
            psb = pp.tile([P, 512], f32, tag="mm")
            nc.tensor.matmul(
                psb, lhsT=ones128_t, rhs=tsums.pop(nn), start=True, stop=True,
            )
            with nc.allow_low_precision(
                reason="softmax denominators; fp16 rel err ~5e-4 is immaterial"
            ):
                nc.vector.reciprocal(rinvR_t[:, sl], psb)
            it = int_ch.pop(nn)
            for si in range(NSK):
                e = eng[0 if si < 10 else 1]
                e.tensor_tensor(expT_t[:, si, sl], expT_t[:, si, sl],
                                rinvR_t[:, sl], Alu.mult)
                e.tensor_tensor(expT_t[:, si, sl], expT_t[:, si, sl],
                                it[:, si, :], Alu.add)

        scores_chunk(0)
        reduce_emit(0)
        scores_chunk(1, mid=lambda: softmax_finish(0))

        # k/q dead after the score matmuls: release before the FFN/out-proj
        # weights go into SBUF. The preload DMAs all run on SP's hardware DGE
        # behind the attention matmuls; ACT keeps doing exps.
        pkq.release()
        pw2 = tc.alloc_tile_pool(name="pW2", bufs=1)
        pw1 = tc.alloc_tile_pool(name="pW1", bufs=1)
        pwo = tc.alloc_tile_pool(name="pWo", bufs=1)
        pln = tc.alloc_tile_pool(name="pLN", bufs=1)
        w2_t = pw2.tile([P, NF, D], f8, tag="w2")
        w2_ap = w2_d.rearrange("(o p) n -> p o n", p=P)
        nc.sync.dma_start(w2_t, w2_ap)
        w1_t = pw1.tile([P, NF, NK, P], f8, tag="w1")
        w1_ap = w1_d.rearrange("f p k q -> p f k q")
        for oc in range(4):
            nc.sync.dma_start(w1_t[:, oc * 8:(oc + 1) * 8, :, :],
                              w1_ap[:, oc * 8:(oc + 1) * 8, :, :])
        wo_t = pwo.tile([P, NK, D], f16, tag="wo")
        nc.sync.dma_start(wo_t, wsl(wo_d))
        g1r_t = pln.tile([P, D], f16, tag="g1r")
        nc.sync.dma_start(g1r_t, g1r_d[:, :])
        b2r_t = pln.tile([P, D], f16, tag="b2r")
        nc.sync.dma_start(b2r_t, b2r_d[:, :])

        pav = tc.alloc_tile_pool(name="pAV", bufs=1)
        avT_t = pav.tile([P, NK, SQ], f16, tag="avT")

        def av_chunk(nn, mid=None):
            """AV^T [d, sq] for one sq chunk: v stationary, attn^T moving."""
            sl = slice(nn * 512, (nn + 1) * 512)
            for mo in range(NK):
                if mid is not None and mo == 3:
                    mid()
                ps = pp.tile([P, 512], f32, tag="mm")
                for si in range(NSK):
                    nc.tensor.matmul(
                        ps,
                        lhsT=v_t[:, si, mo * P:(mo + 1) * P],
                        rhs=expT_t[:, si, sl],
                        start=(si == 0),
                        stop=(si == NSK - 1),
                    )
                nc.scalar.copy(avT_t[:, mo, sl], ps)

        int_fetch(1)
        reduce_emit(1)
        av_chunk(0, mid=lambda: softmax_finish(1))
        av_chunk(1)

        pint.release()
        pe.release()
        pv.release()

        # ============ out-proj + residual + LN1 + h1^T (pipelined) ===========
        ph1 = tc.alloc_tile_pool(name="pH1", bufs=1)
        ph1t = tc.alloc_tile_pool(name="pH1T", bufs=1)
        pxh = tc.alloc_tile_pool(name="pXh", bufs=3)

        h1_t = ph1.tile([P, NQT, D], f16, tag="h1")
        h1T_h = [
            ph1t.tile([P, NK, 512], f8, tag="h1T0", name="h1T_0"),
            ph1t.tile([P, NK, 512], f8, tag="h1T1", name="h1T_1"),
        ]

        def outproj(st_):
            """hin[st_] = avT^T @ Wo + (X + bo), fp32."""
            xh = pxh.tile([P, D], f16, tag="xh")
            nc.gpsimd.dma_start(xh, xh_d[st_ * P:(st_ + 1) * P, :])
            hin = pxh.tile([P, D], f32, tag="hin", name=f"hin_{st_}")
            for nn in range(D // 512):
                ps = pp.tile([P, 512], f32, tag="mm")
                for mo in range(NK):
                    nc.tensor.matmul(
                        ps,
                        lhsT=avT_t[:, mo, st_ * P:(st_ + 1) * P],
                        rhs=wo_t[:, mo, nn * 512:(nn + 1) * 512],
                        start=(mo == 0),
                        stop=(mo == NK - 1),
                    )
                nc.vector.tensor_tensor(
                    hin[:, nn * 512:(nn + 1) * 512], ps,
                    xh[:, nn * 512:(nn + 1) * 512], Alu.add,
                )
            return hin

        zs = {}

        def ln1(st_, hin):
            """LayerNorm stats + z (DVE); h1 = 512*(z*g1 + b2 + be1) in f16."""
            st = sp.tile([P, 2, 6], f32, tag="bst")
            nc.vector.bn_stats(st[:, 0, :], hin[:, 0:512])
            nc.vector.bn_stats(st[:, 1, :], hin[:, 512:1024])
            mv = sp.tile([P, 2], f32, tag="mv")
            nc.vector.bn_aggr(mv, st)
            sd = sp.tile([P, 1], f32, tag="sd")
            nc.scalar.activation(sd, mv[:, 1:2], Act.Sqrt, bias=eps_t, scale=1.0)
            rstd = sp.tile([P, 1], f32, tag="rstd")
            nc.vector.reciprocal(rstd, sd)
            nmr = sp.tile([P, 1], f32, tag="nmr")
            nc.vector.tensor_scalar(nmr, mv[:, 0:1], rstd, -1.0,
                                    Alu.mult, Alu.mult)
            z = sp.tile([P, D], f16, tag="z16", bufs=3, name=f"z_{st_}")
            nc.scalar.activation(z, hin, Act.Identity, bias=nmr, scale=rstd)
            zs[st_] = z
            # h1 carries the 512x descale and the (b2+be1) row for FFN2
            ho = h1_t[:, st_, :]
            nc.gpsimd.tensor_tensor(ho, z, g1r_t, Alu.mult)
            nc.gpsimd.tensor_tensor(ho, ho, b2r_t, Alu.add)

        def transpose_z(st_):
            """h1T tiles (f8) for FFN1 via PE transposes of z."""
            z = zs.pop(st_)
            half, stl = divmod(st_, 4)
            for di in range(NK):
                tp = pps.tile([P, P], f16, tag="tp", bufs=2, name="tp")
                nc.tensor.transpose(tp, z[:, di * P:(di + 1) * P], ident_t)
                dst = h1T_h[half][:, di, stl * P:(stl + 1) * P]
                if di % 2 == 0:
                    nc.scalar.copy(dst, tp)
                else:
                    nc.gpsimd.tensor_copy(out=dst, in_=tp)

        # ================= phase C helpers: FFN ==============================
        pffn = tc.alloc_tile_pool(name="pFFN", bufs=2)
        pout = tc.alloc_tile_pool(name="pOut", bufs=2)
        f1T_h = {}

        def ffn1(half):
            """f1^T [f, sq-half] = 8*leaky(z^T @ W1p + b1p) in fp8 DR.
            Evacuation split: t16 on ACT (even fo) / DVE (odd fo); the leaky
            max as one scalar_tensor_tensor on DVE (even) / Pool (odd)."""
            f1T_t = pffn.tile([P, NF, 512], f8, tag="f1T", name=f"f1T_{half}")
            f1T_h[half] = f1T_t
            for fo in range(NF):
                ps = pp.tile([P, 512], f32, tag="mm")
                for di in range(0, NK, 2):
                    nc.tensor.matmul(
                        ps,
                        lhsT=w1_t[:, fo, di:di + 2, :],
                        rhs=h1T_h[half][:, di:di + 2, :],
                        start=(di == 0),
                        stop=(di == NK - 2),
                        perf_mode=DR,
                    )
                t16 = pout.tile([P, 512], f16, tag="t16", bufs=3)
                if fo % 3 != 2:
                    nc.scalar.activation(
                        t16, ps, Act.Identity, bias=b1p_t[:, fo:fo + 1],
                        scale=0.03125,
                    )
                else:
                    nc.vector.tensor_scalar(
                        t16, ps, 0.03125, b1p_t[:, fo:fo + 1],
                        Alu.mult, Alu.add,
                    )
                eng[1 if fo % 3 != 1 else 0].scalar_tensor_tensor(
                    f1T_t[:, fo, :], t16, SLOPE, t16, Alu.mult, Alu.max,
                )

        def ffn2(half):
            """hin2 = f1^T^T @ W2 + h1 (all 512-scaled), then LN2 -> out."""
            f1T_t = f1T_h.pop(half)
            for stl in range(4):
                st_ = half * 4 + stl
                hin = pout.tile([P, D], f16, tag="hin2")
                st2 = sp.tile([P, 2, 6], f32, tag="bst")
                for nn in range(D // 512):
                    sl = slice(nn * 512, (nn + 1) * 512)
                    ps = pp.tile([P, 512], f32, tag="mm")
                    for fi in range(0, NF, 2):
                        nc.tensor.matmul(
                            ps,
                            lhsT=f1T_t[:, fi:fi + 2, stl * P:(stl + 1) * P],
                            rhs=w2_t[:, fi:fi + 2, nn * 512:(nn + 1) * 512],
                            start=(fi == 0),
                            stop=(fi == NF - 2),
                            perf_mode=DR,
                        )
                    nc.vector.tensor_tensor(
                        hin[:, sl], ps, h1_t[:, st_, sl], Alu.add,
                    )
                    nc.vector.bn_stats(st2[:, nn, :], hin[:, sl])
                mv = sp.tile([P, 2], f32, tag="mv")
                nc.vector.bn_aggr(mv, st2)
                sd = sp.tile([P, 1], f32, tag="sd")
                nc.scalar.activation(sd, mv[:, 1:2], Act.Sqrt, bias=eps_t,
                                     scale=1.0)
                rstd = sp.tile([P, 1], f32, tag="rstd")
                nc.vector.reciprocal(rstd, sd)
                nmr = sp.tile([P, 1], f32, tag="nmr")
                nc.vector.tensor_scalar(nmr, mv[:, 0:1], rstd, -1.0,
                                        Alu.mult, Alu.mult)
                zo = pout.tile([P, D], f16, tag="zout")
                for ch in range(2):
                    sl = slice(ch * 512, (ch + 1) * 512)
                    # out = normalized(hin); the g2/be2 affine is applied on
                    # the host (it's the last op, nothing downstream on-chip)
                    nc.scalar.activation(zo[:, sl], hin[:, sl], Act.Identity,
                                         bias=nmr, scale=rstd)
                    nc.sync.dma_start(out_d[st_ * P:(st_ + 1) * P, sl],
                                      zo[:, sl])

        # ---- pipelined emission: transposes hide behind the next out-proj;
        # FFN2(half 0) interposes before the last transpose so z7's LN chain
        # has a full matmul block to complete under.
        hins = {}
        hins[0] = outproj(0)
        hins[1] = outproj(1)
        ln1(0, hins.pop(0))
        hins[2] = outproj(2)
        ln1(1, hins.pop(1))
        transpose_z(0)
        hins[3] = outproj(3)
        ln1(2, hins.pop(2))
        transpose_z(1)
        hins[4] = outproj(4)
        ln1(3, hins.pop(3))
        transpose_z(2)
        hins[5] = outproj(5)
        ln1(4, hins.pop(4))
        transpose_z(3)
        ffn1(0)
        hins[6] = outproj(6)
        ln1(5, hins.pop(5))
        transpose_z(4)
        hins[7] = outproj(7)
        ln1(6, hins.pop(6))
        transpose_z(5)
        ln1(7, hins.pop(7))
        ffn2(0)
        transpose_z(6)
        transpose_z(7)
        ffn1(1)
        ffn2(1)

        pout.release()
        pffn.release()
        pxh.release()
        ph1t.release()
        ph1.release()
        pav.release()
        pln.release()
        pwo.release()
        pw1.release()
        pw2.release()
        sp.release()
        pps.release()
        pp.release()
        cp.release()

    nc.finalize()
    return nc


def _host_prep(inputs):
    import ml_dtypes
    f16 = np.float16
    f32 = np.float32
    f8 = ml_dtypes.float8_e4m3fn
    X = np.asarray(inputs["X"], f32)
    I = np.asarray(inputs["intensity"], f32)

    W1 = np.asarray(inputs["W1"], np.float64)
    g1 = np.asarray(inputs["g1"], np.float64)
    be1 = np.asarray(inputs["be1"], np.float64)
    W1p = (W1 * g1[:, None]).astype(np.float32)
    b1p = (np.asarray(inputs["b1"], np.float64) + be1 @ W1).astype(np.float32)
    # fp8 weights with power-of-2 prescales; see the module docstring
    w1t4 = np.ascontiguousarray(
        (256.0 * W1p).astype(f8).reshape(NK, P, NF, P).transpose(2, 1, 0, 3)
    )
    shared = {
        "wq8": np.asarray(inputs["Wq"], np.float32).astype(f8),
        "wk8": np.asarray(inputs["Wk"], np.float32).astype(f8),
        "wv": np.asarray(inputs["Wv"], f16),
        "wo": np.asarray(inputs["Wo"], f16),
        "w1t4": w1t4,
        "w2": (64.0 * np.asarray(inputs["W2"], np.float32)).astype(f8),
        "bq_p": np.ascontiguousarray(np.asarray(inputs["bq"], f32).reshape(NK, P).T),
        "bk_p": np.ascontiguousarray(np.asarray(inputs["bk"], f32).reshape(NK, P).T),
        "bvr": np.ascontiguousarray(
            np.broadcast_to(np.asarray(inputs["bv"], f16)[None, :], (P, D))
        ),
        "b1_p": np.ascontiguousarray((8.0 * b1p).reshape(NF, P).T),
        "b2r": np.ascontiguousarray(np.broadcast_to(
            (512.0 * (np.asarray(inputs["b2"], np.float64)
                      + np.asarray(inputs["be1"], np.float64))
             ).astype(f16)[None, :],
            (P, D))),
        "g1r": np.ascontiguousarray(np.broadcast_to(
            (512.0 * np.asarray(inputs["g1"], np.float64)).astype(f16)[None, :],
            (P, D))),
    }

    in_maps = []
    for c in range(8):
        b, h = divmod(c, 2)
        own = slice(h * SQ, (h + 1) * SQ)
        oth = slice((1 - h) * SQ, (2 - h) * SQ)
        # sk order: own query rows first, then the other half, so q^T is a
        # contiguous slice of X^T. intensity columns follow the same order.
        xb = np.concatenate([X[b, own], X[b, oth]], axis=0)
        Ih = I[b, own]
        intT = np.concatenate([Ih[:, own], Ih[:, oth]], axis=1).T
        m = dict(shared)
        xbT = np.ascontiguousarray(xb.T.astype(f16))
        m["xbT"] = xbT
        m["xbT8"] = xbT.astype(f8)
        m["xh16"] = (X[b, own] + np.asarray(inputs["bo"], f32)[None, :]).astype(f16)
        m["intT"] = np.ascontiguousarray(intT.astype(f16))
        in_maps.append(m)
    return in_maps


def kernel(**inputs) -> np.ndarray:
    global _PROG
    if _PROG is None:
        _PROG = _build()
    from concourse.bass_utils import run_bass_kernel_spmd

    in_maps = _host_prep(inputs)
    res = run_bass_kernel_spmd(_PROG, in_maps, list(range(8)))
    out = np.empty((B, S, D), np.float32)
    for c, r in enumerate(res.results):
        b, h = divmod(c, 2)
        out[b, h * SQ:(h + 1) * SQ] = r["out"]
    g2 = np.asarray(inputs["g2"], np.float32)
    be2 = np.asarray(inputs["be2"], np.float32)
    return out * g2 + be2


# revision 32
# speedup vs baseline: 1.1018x; 1.0077x over previous
"""Trainium2 Bass kernel for a transformer encoder layer (B=4, S=2048, D=1024, DFF=4096).

Sharding: data-parallel, no collectives. Core c = 2*b + h handles query rows
[b, h*1024:(h+1)*1024]. Each core computes K/V for its full batch (the pair of
cores sharing a batch duplicate that work).

Layout strategy: all attention math runs in "transposed" layouts so no on-device
transposes are needed:
  - X^T via DMA-transpose (host provides fp16/fp8 X),
  - scores computed as scores^T [sk, sq] (k^T stationary, q^T moving),
  - softmax sums over sk (partitions) via a DVE/Pool add-tree + one ones-matmul,
  - intensity supplied pre-transposed by the host,
  - AV^T [d, sq] comes out of the PE directly in the layout the out-proj needs,
  - h1^T (pre-affine z^T) via PE transposes of 128x128 tiles.

Precision: QK/scores and the whole FFN run fp8 DoubleRow (2 rows/cycle); W1/W2
carry power-of-2 prescales (x256 / x64) so their U(-1/32..) ranges land in
e4m3's normal range; the scales come out in the evacuations (f1 is stored at
8x, the FFN2 psum and the h1 residual at 512x -- LayerNorm is scale-invariant).
V/AV/out-proj stay fp16 (their error feeds the residual stream unattenuated).

Scheduling: the PE instruction stream is ordered so every long-latency chain
(softmax denominators, LN1 -> z -> transpose, FFN1 -> f1 evac) is hidden behind
unrelated matmuls; evacuation work is spread across ACT/DVE/Pool so no single
engine gates the PE.
"""

import sys

if "/opt/trn_rl_repo" not in sys.path:
    sys.path.insert(0, "/opt/trn_rl_repo")

import numpy as np

P = 128
B, S, D, DFF = 4, 2048, 1024, 4096
SQ = 1024                 # query rows per core
NK = D // P               # 8  d tiles
NSK = S // P              # 16 sk tiles
NF = DFF // P             # 32 f tiles
NQT = SQ // P             # 8  sq tiles
EPS = 1e-6
SLOPE = 0.01
SCALE = 1.0 / 32.0        # 1/sqrt(D)

_PROG = None


def _build():
    import concourse.mybir as mybir
    import concourse.tile as tile
    from concourse import bacc

    f16 = mybir.dt.float16
    f32 = mybir.dt.float32
    f8 = mybir.dt.float8e4
    Act = mybir.ActivationFunctionType
    Alu = mybir.AluOpType
    DR = mybir.MatmulPerfMode.DoubleRow

    nc = bacc.Bacc("TRN2", debug=False)

    # ---- I/O ----------------------------------------------------------------
    xbT_d = nc.dram_tensor("xbT", [D, S], f16, kind="ExternalInput")
    xbT8_d = nc.dram_tensor("xbT8", [D, S], f8, kind="ExternalInput")
    xh_d = nc.dram_tensor("xh16", [SQ, D], f16, kind="ExternalInput")
    intT_d = nc.dram_tensor("intT", [S, SQ], f16, kind="ExternalInput")
    wq_d = nc.dram_tensor("wq8", [D, D], f8, kind="ExternalInput")
    wk_d = nc.dram_tensor("wk8", [D, D], f8, kind="ExternalInput")
    wv_d = nc.dram_tensor("wv", [D, D], f16, kind="ExternalInput")
    wo_d = nc.dram_tensor("wo", [D, D], f16, kind="ExternalInput")
    # W1 pre-tiled on host to [NF, P(d_in part), NK, P(f)] for contiguous DMA
    w1_d = nc.dram_tensor("w1t4", [NF, P, NK, P], f8, kind="ExternalInput")
    w2_d = nc.dram_tensor("w2", [DFF, D], f8, kind="ExternalInput")
    bq_d = nc.dram_tensor("bq_p", [P, NK], f32, kind="ExternalInput")
    bk_d = nc.dram_tensor("bk_p", [P, NK], f32, kind="ExternalInput")
    bvr_d = nc.dram_tensor("bvr", [P, D], f16, kind="ExternalInput")
    b1p_d = nc.dram_tensor("b1_p", [P, NF], f32, kind="ExternalInput")
    b2r_d = nc.dram_tensor("b2r", [P, D], f16, kind="ExternalInput")
    g1r_d = nc.dram_tensor("g1r", [P, D], f16, kind="ExternalInput")
    out_d = nc.dram_tensor("out", [SQ, D], f16, kind="ExternalOutput")

    def wsl(wd):
        # [D, N] dram -> [P, NK, N] AP (partition-major tiles of contraction dim)
        return wd.rearrange("(o p) n -> p o n", p=P)

    with tile.TileContext(nc) as tc:
        # ---- long-lived pools ----
        cp = tc.alloc_tile_pool(name="consts", bufs=1)
        pp = tc.alloc_tile_pool(name="psum", bufs=6, space="PSUM")
        pps = tc.alloc_tile_pool(name="psrow", bufs=2, space="PSUM")
        sp = tc.alloc_tile_pool(name="stats", bufs=2)

        ident_t = cp.tile([P, P], f16, tag="ident")
        from concourse.masks import make_identity
        make_identity(nc, ident_t)
        ones128_t = cp.tile([P, P], f16, tag="ones128")
        nc.vector.memset(ones128_t, 1.0)
        rinvR_t = cp.tile([P, SQ], f16, tag="rinvR")
        eps_t = cp.tile([P, 1], f32, tag="eps")
        nc.vector.memset(eps_t, EPS)

        # ================= phase A: X^T, k^T, q^T, v =========================
        pv = tc.alloc_tile_pool(name="pV", bufs=1, side="right")
        pkq = tc.alloc_tile_pool(name="pKQ", bufs=1)
        pxt = tc.alloc_tile_pool(name="pXT", bufs=1)
        pw = tc.alloc_tile_pool(name="pW", bufs=2)
        pxt8 = tc.alloc_tile_pool(name="pXT8", bufs=1)

        xT_t = pxt.tile([P, NK, S], f16, tag="xT")
        xbT_ap = xbT_d.rearrange("(o p) s -> p o s", p=P)
        xT8_t = pxt8.tile([P, NK, S], f8, tag="xT8")
        xbT8_ap = xbT8_d.rearrange("(o p) s -> p o s", p=P)

        kT_t = pkq.tile([P, NK, S], f8, tag="kT")
        qT_t = pkq.tile([P, NK, SQ], f8, tag="qT")
        v_t = pv.tile([P, NSK, D], f16, tag="v")

        # k^T [d_out, sk] = Wk^T @ X^T in fp8 DoubleRow (softmax absorbs the
        # quantization; bias fused into the ACT evacuation)
        wk_t = pw.tile([P, NK, D], f8, tag="wmat8")
        wk_ap = wsl(wk_d)
        # DMAs occupy their issuing engine for the whole transfer in this
        # machine model, so spread: SP takes wk (needed first; ACT is stuck
        # loading its activation table at t=0) then the q/v weights; Pool
        # streams X^T fp8 then fp16; ACT only does the small bias rows.
        nc.sync.dma_start(wk_t[:, 0:4, :], wk_ap[:, 0:4, :])
        nc.gpsimd.dma_start(wk_t[:, 4:8, :], wk_ap[:, 4:8, :])
        for nn in range(S // 512):
            [nc.scalar, nc.sync, nc.gpsimd, nc.sync][nn].dma_start(
                xT8_t[:, :, nn * 512:(nn + 1) * 512],
                xbT8_ap[:, :, nn * 512:(nn + 1) * 512])
        bq_t = cp.tile([P, NK], f32, tag="bq")
        nc.scalar.dma_start(bq_t, bq_d[:, :])
        bk_t = cp.tile([P, NK], f32, tag="bk")
        nc.scalar.dma_start(bk_t, bk_d[:, :])
        bvr_t = cp.tile([P, D], f16, tag="bvr")
        nc.scalar.dma_start(bvr_t, bvr_d[:, :])
        b1p_t = cp.tile([P, NF], f32, tag="b1p")
        nc.scalar.dma_start(b1p_t, b1p_d[:, :])
        for nn in range(S // 512):
            for mo in range(NK):
                ps = pp.tile([P, 512], f32, tag="mm")
                for dj in range(0, NK, 2):
                    nc.tensor.matmul(
                        ps,
                        lhsT=wk_t[:, dj:dj + 2, mo * P:(mo + 1) * P],
                        rhs=xT8_t[:, dj:dj + 2, nn * 512:(nn + 1) * 512],
                        start=(dj == 0),
                        stop=(dj == NK - 2),
                        perf_mode=DR,
                    )
                if mo % 2 == 0:
                    nc.scalar.activation(
                        kT_t[:, mo, nn * 512:(nn + 1) * 512], ps,
                        Act.Identity, bias=bk_t[:, mo:mo + 1], scale=1.0,
                    )
                else:
                    nc.vector.tensor_scalar(
                        kT_t[:, mo, nn * 512:(nn + 1) * 512], ps,
                        bk_t[:, mo:mo + 1], None, Alu.add,
                    )

        # q^T [d_out, sq]  (this core's rows = first SQ columns of X^T)
        wq_t = pw.tile([P, NK, D], f8, tag="wmat8")
        nc.sync.dma_start(xT_t[:, 0:2, :], xbT_ap[:, 0:2, :])
        nc.sync.dma_start(wq_t, wsl(wq_d))
        for oc in range(3):
            nc.gpsimd.dma_start(xT_t[:, 2 + 2 * oc:4 + 2 * oc, :],
                                xbT_ap[:, 2 + 2 * oc:4 + 2 * oc, :])
        for mo in range(NK):
            for nn in range(SQ // 512):
                ps = pp.tile([P, 512], f32, tag="mm")
                for dj in range(0, NK, 2):
                    nc.tensor.matmul(
                        ps,
                        lhsT=wq_t[:, dj:dj + 2, mo * P:(mo + 1) * P],
                        rhs=xT8_t[:, dj:dj + 2, nn * 512:(nn + 1) * 512],
                        start=(dj == 0),
                        stop=(dj == NK - 2),
                        perf_mode=DR,
                    )
                nc.vector.tensor_scalar(
                    qT_t[:, mo, nn * 512:(nn + 1) * 512], ps,
                    bq_t[:, mo:mo + 1], None, Alu.add,
                )

        # v [sk, d] = X @ Wv + bv, interleaved with chunk 0's softmax: the
        # score matmuls for chunk 0 run BEFORE v, so the 27us of v matmuls
        # hide the exp evacuations (ACT is slower than the PE there), the
        # denominator reduce, and the chunk-0 normalize. v's evacuations are
        # split DVE/Pool by si parity to keep both under the PE rate.
        wv_t = pw.tile([P, NK, D], f16, tag="wmat", bufs=1)
        nc.gpsimd.dma_start(wv_t, wsl(wv_d))

        def v_compute(mid=None):
            for si in range(NSK):
                if mid is not None and si == 8:
                    mid()
                for nn in range(D // 512):
                    ps = pp.tile([P, 512], f32, tag="mm")
                    for di in range(NK):
                        nc.tensor.matmul(
                            ps,
                            lhsT=xT_t[:, di, si * P:(si + 1) * P],
                            rhs=wv_t[:, di, nn * 512:(nn + 1) * 512],
                            start=(di == 0),
                            stop=(di == NK - 1),
                        )
                    [nc.vector, nc.gpsimd][si % 2].tensor_tensor(
                        v_t[:, si, nn * 512:(nn + 1) * 512], ps,
                        bvr_t[:, nn * 512:(nn + 1) * 512], Alu.add,
                    )

        # ================= phase B: attention ================================
        pxt8.release()
        pe = tc.alloc_tile_pool(name="pE", bufs=1, side="right")
        pint = tc.alloc_tile_pool(name="pInt", bufs=1, side="right")
        expT_t = pe.tile([P, NSK, SQ], f16, tag="expT")
        intT_ap = intT_d.rearrange("(si p) q -> p si q", p=P)

        int_ch = {}

        def int_fetch(nn):
            # bufs=1: chunk 1's DMA implicitly waits for chunk 0's reads
            it = pint.tile([P, NSK, 512], f16, tag="intT")
            nc.sync.dma_start(it, intT_ap[:, :, nn * 512:(nn + 1) * 512])
            int_ch[nn] = it

        int_fetch(0)

        # scores^T [sk, sq] with exp(s/32) fused into the PSUM evacuation
        def scores_chunk(nn, mid=None):
            sl = slice(nn * 512, (nn + 1) * 512)
            for si in range(NSK):
                if mid is not None and si == 5:
                    mid()
                ps = pp.tile([P, 512], f32, tag="mm")
                for dj in range(0, NK, 2):
                    nc.tensor.matmul(
                        ps,
                        lhsT=kT_t[:, dj:dj + 2, si * P:(si + 1) * P],
                        rhs=qT_t[:, dj:dj + 2, sl],
                        start=(dj == 0),
                        stop=(dj == NK - 2),
                        perf_mode=DR,
                    )
                nc.scalar.activation(
                    expT_t[:, si, sl], ps, Act.Exp, bias=0.0, scale=SCALE,
                )

        eng = [nc.vector, nc.gpsimd]
        tsums = {}

        def reduce_emit(nn):
            """Denominator partials: DVE X-reduces si 0-3 / 4-7 (start as soon
            as those exps land, during the score matmuls); Pool pairwise-adds
            si 8-15; two DVE adds merge."""
            sl = slice(nn * 512, (nn + 1) * 512)
            tsum = sp.tile([P, 512], f16, tag="dsum")
            tsB = sp.tile([P, 512], f16, tag="dsB")
            tp4 = sp.tile([P, 4, 512], f16, tag="dp4", bufs=1)
            tp2 = sp.tile([P, 2, 512], f16, tag="dp2", bufs=1)
            tpb = sp.tile([P, 512], f16, tag="dpb", bufs=1)
            for j in range(4):
                nc.gpsimd.tensor_tensor(
                    tp4[:, j, :], expT_t[:, 8 + 2 * j, sl],
                    expT_t[:, 9 + 2 * j, sl], Alu.add,
                )
            for j in range(2):
                nc.gpsimd.tensor_tensor(
                    tp2[:, j, :], tp4[:, 2 * j, :], tp4[:, 2 * j + 1, :],
                    Alu.add,
                )
            nc.gpsimd.tensor_tensor(tpb, tp2[:, 0, :], tp2[:, 1, :], Alu.add)
            with nc.allow_low_precision(
                reason="softmax denominators; fp16 rel err ~5e-4 is immaterial"
            ):
                nc.vector.tensor_reduce(
                    tsum, expT_t[:, 0:4, sl].rearrange("p a b -> p b a"),
                    mybir.AxisListType.X, Alu.add,
                )
                nc.vector.tensor_reduce(
                    tsB, expT_t[:, 4:8, sl].rearrange("p a b -> p b a"),
                    mybir.AxisListType.X, Alu.add,
                )
            nc.vector.tensor_tensor(tsum, tsum, tsB, Alu.add)
            nc.vector.tensor_tensor(tsum, tsum, tpb, Alu.add)
            tsums[nn] = tsum

        def softmax_finish(nn):
            """Broadcast the partition-sum via a ones-matmul (PE), reciprocal,
            then normalize + add intensity in place; si<10 on DVE, rest on
            Pool, matching AV's in-order consumption."""
            sl = slice(nn * 512, (nn + 1) * 512)
            psb = pp.tile([P, 512], f32, tag="mm")
            nc.tensor.matmul(
                psb, lhsT=ones128_t, rhs=tsums.pop(nn), start=True, stop=True,
            )
            with nc.allow_low_precision(
                reason="softmax denominators; fp16 rel err ~5e-4 is immaterial"
            ):
                nc.vector.reciprocal(rinvR_t[:, sl], psb)
            it = int_ch.pop(nn)
            for si in range(NSK):
                e = eng[0 if si < 10 else 1]
                e.tensor_tensor(expT_t[:, si, sl], expT_t[:, si, sl],
                                rinvR_t[:, sl], Alu.mult)
                e.tensor_tensor(expT_t[:, si, sl], expT_t[:, si, sl],
                                it[:, si, :], Alu.add)

        scores_chunk(0)
        reduce_emit(0)

        def _mid_v():
            softmax_finish(0)
            int_fetch(1)

        v_compute(mid=_mid_v)
        pw.release()
        pxt.release()
        scores_chunk(1)
        reduce_emit(1)

        # k/q dead after the score matmuls: release before the FFN/out-proj
        # weights go into SBUF. The preload DMAs all run on SP's hardware DGE
        # behind the attention matmuls; ACT keeps doing exps.
        pkq.release()
        pw2 = tc.alloc_tile_pool(name="pW2", bufs=1)
        pw1 = tc.alloc_tile_pool(name="pW1", bufs=1)
        pwo = tc.alloc_tile_pool(name="pWo", bufs=1)
        pln = tc.alloc_tile_pool(name="pLN", bufs=1)
        w2_t = pw2.tile([P, NF, D], f8, tag="w2")
        w2_ap = w2_d.rearrange("(o p) n -> p o n", p=P)
        nc.sync.dma_start(w2_t, w2_ap)
        w1_t = pw1.tile([P, NF, NK, P], f8, tag="w1")
        w1_ap = w1_d.rearrange("f p k q -> p f k q")
        for oc in range(4):
            nc.sync.dma_start(w1_t[:, oc * 8:(oc + 1) * 8, :, :],
                              w1_ap[:, oc * 8:(oc + 1) * 8, :, :])
        wo_t = pwo.tile([P, NK, D], f16, tag="wo")
        nc.sync.dma_start(wo_t, wsl(wo_d))
        g1r_t = pln.tile([P, D], f16, tag="g1r")
        nc.sync.dma_start(g1r_t, g1r_d[:, :])
        b2r_t = pln.tile([P, D], f16, tag="b2r")
        nc.sync.dma_start(b2r_t, b2r_d[:, :])

        pav = tc.alloc_tile_pool(name="pAV", bufs=1)
        avT_t = pav.tile([P, NK, SQ], f16, tag="avT")

        def av_chunk(nn, mid=None):
            """AV^T [d, sq] for one sq chunk: v stationary, attn^T moving."""
            sl = slice(nn * 512, (nn + 1) * 512)
            for mo in range(NK):
                if mid is not None and mo == 3:
                    mid()
                ps = pp.tile([P, 512], f32, tag="mm")
                for si in range(NSK):
                    nc.tensor.matmul(
                        ps,
                        lhsT=v_t[:, si, mo * P:(mo + 1) * P],
                        rhs=expT_t[:, si, sl],
                        start=(si == 0),
                        stop=(si == NSK - 1),
                    )
                nc.scalar.copy(avT_t[:, mo, sl], ps)

        av_chunk(0, mid=lambda: softmax_finish(1))
        av_chunk(1)

        pint.release()
        pe.release()
        pv.release()

        # ============ out-proj + residual + LN1 + h1^T (pipelined) ===========
        ph1 = tc.alloc_tile_pool(name="pH1", bufs=1)
        ph1t = tc.alloc_tile_pool(name="pH1T", bufs=1)
        pxh = tc.alloc_tile_pool(name="pXh", bufs=3)

        h1_t = ph1.tile([P, NQT, D], f16, tag="h1")
        h1T_h = [
            ph1t.tile([P, NK, 512], f8, tag="h1T0", name="h1T_0"),
            ph1t.tile([P, NK, 512], f8, tag="h1T1", name="h1T_1"),
        ]

        def outproj(st_):
            """hin[st_] = avT^T @ Wo + (X + bo), fp32."""
            xh = pxh.tile([P, D], f16, tag="xh")
            nc.gpsimd.dma_start(xh, xh_d[st_ * P:(st_ + 1) * P, :])
            hin = pxh.tile([P, D], f32, tag="hin", name=f"hin_{st_}")
            for nn in range(D // 512):
                ps = pp.tile([P, 512], f32, tag="mm")
                for mo in range(NK):
                    nc.tensor.matmul(
                        ps,
                        lhsT=avT_t[:, mo, st_ * P:(st_ + 1) * P],
                        rhs=wo_t[:, mo, nn * 512:(nn + 1) * 512],
                        start=(mo == 0),
                        stop=(mo == NK - 1),
                    )
                nc.vector.tensor_tensor(
                    hin[:, nn * 512:(nn + 1) * 512], ps,
                    xh[:, nn * 512:(nn + 1) * 512], Alu.add,
                )
            return hin

        zs = {}

        def ln1(st_, hin):
            """LayerNorm stats + z (DVE); h1 = 512*(z*g1 + b2 + be1) in f16."""
            st = sp.tile([P, 2, 6], f32, tag="bst")
            nc.vector.bn_stats(st[:, 0, :], hin[:, 0:512])
            nc.vector.bn_stats(st[:, 1, :], hin[:, 512:1024])
            mv = sp.tile([P, 2], f32, tag="mv")
            nc.vector.bn_aggr(mv, st)
            sd = sp.tile([P, 1], f32, tag="sd")
            nc.scalar.activation(sd, mv[:, 1:2], Act.Sqrt, bias=eps_t, scale=1.0)
            rstd = sp.tile([P, 1], f32, tag="rstd")
            nc.vector.reciprocal(rstd, sd)
            nmr = sp.tile([P, 1], f32, tag="nmr")
            nc.vector.tensor_scalar(nmr, mv[:, 0:1], rstd, -1.0,
                                    Alu.mult, Alu.mult)
            z = sp.tile([P, D], f16, tag="z16", bufs=3, name=f"z_{st_}")
            nc.scalar.activation(z, hin, Act.Identity, bias=nmr, scale=rstd)
            zs[st_] = z
            # h1 carries the 512x descale and the (b2+be1) row for FFN2
            ho = h1_t[:, st_, :]
            nc.gpsimd.tensor_tensor(ho, z, g1r_t, Alu.mult)
            nc.gpsimd.tensor_tensor(ho, ho, b2r_t, Alu.add)

        def transpose_z(st_):
            """h1T tiles (f8) for FFN1 via PE transposes of z."""
            z = zs.pop(st_)
            half, stl = divmod(st_, 4)
            for di in range(NK):
                tp = pps.tile([P, P], f16, tag="tp", bufs=2, name="tp")
                nc.tensor.transpose(tp, z[:, di * P:(di + 1) * P], ident_t)
                dst = h1T_h[half][:, di, stl * P:(stl + 1) * P]
                if di % 2 == 0:
                    nc.scalar.copy(dst, tp)
                else:
                    nc.gpsimd.tensor_copy(out=dst, in_=tp)

        # ================= phase C helpers: FFN ==============================
        pffn = tc.alloc_tile_pool(name="pFFN", bufs=2)
        pout = tc.alloc_tile_pool(name="pOut", bufs=2)
        f1T_h = {}

        def ffn1(half):
            """f1^T [f, sq-half] = 8*leaky(z^T @ W1p + b1p) in fp8 DR.
            Evacuation split: t16 on ACT (even fo) / DVE (odd fo); the leaky
            max as one scalar_tensor_tensor on DVE (even) / Pool (odd)."""
            f1T_t = pffn.tile([P, NF, 512], f8, tag="f1T", name=f"f1T_{half}")
            f1T_h[half] = f1T_t
            for fo in range(NF):
                ps = pp.tile([P, 512], f32, tag="mm")
                for di in range(0, NK, 2):
                    nc.tensor.matmul(
                        ps,
                        lhsT=w1_t[:, fo, di:di + 2, :],
                        rhs=h1T_h[half][:, di:di + 2, :],
                        start=(di == 0),
                        stop=(di == NK - 2),
                        perf_mode=DR,
                    )
                t16 = pout.tile([P, 512], f16, tag="t16", bufs=3)
                if fo % 3 != 2:
                    nc.scalar.activation(
                        t16, ps, Act.Identity, bias=b1p_t[:, fo:fo + 1],
                        scale=0.03125,
                    )
                else:
                    nc.vector.tensor_scalar(
                        t16, ps, 0.03125, b1p_t[:, fo:fo + 1],
                        Alu.mult, Alu.add,
                    )
                eng[1 if fo % 3 != 1 else 0].scalar_tensor_tensor(
                    f1T_t[:, fo, :], t16, SLOPE, t16, Alu.mult, Alu.max,
                )

        def ffn2(half):
            """hin2 = f1^T^T @ W2 + h1 (all 512-scaled), then LN2 -> out."""
            f1T_t = f1T_h.pop(half)
            for stl in range(4):
                st_ = half * 4 + stl
                hin = pout.tile([P, D], f16, tag="hin2")
                st2 = sp.tile([P, 2, 6], f32, tag="bst")
                for nn in range(D // 512):
                    sl = slice(nn * 512, (nn + 1) * 512)
                    ps = pp.tile([P, 512], f32, tag="mm")
                    for fi in range(0, NF, 2):
                        nc.tensor.matmul(
                            ps,
                            lhsT=f1T_t[:, fi:fi + 2, stl * P:(stl + 1) * P],
                            rhs=w2_t[:, fi:fi + 2, nn * 512:(nn + 1) * 512],
                            start=(fi == 0),
                            stop=(fi == NF - 2),
                            perf_mode=DR,
                        )
                    nc.vector.tensor_tensor(
                        hin[:, sl], ps, h1_t[:, st_, sl], Alu.add,
                    )
                    nc.vector.bn_stats(st2[:, nn, :], hin[:, sl])
                mv = sp.tile([P, 2], f32, tag="mv")
                nc.vector.bn_aggr(mv, st2)
                sd = sp.tile([P, 1], f32, tag="sd")
                nc.scalar.activation(sd, mv[:, 1:2], Act.Sqrt, bias=eps_t,
                                     scale=1.0)
                rstd = sp.tile([P, 1], f32, tag="rstd")
                nc.vector.reciprocal(rstd, sd)
                nmr = sp.tile([P, 1], f32, tag="nmr")
                nc.vector.tensor_scalar(nmr, mv[:, 0:1], rstd, -1.0,
                                        Alu.mult, Alu.mult)
                zo = pout.tile([P, D], f16, tag="zout")
                for ch in range(2):
                    sl = slice(ch * 512, (ch + 1) * 512)
                    # out = normalized(hin); the g2/be2 affine is applied on
                    # the host (it's the last op, nothing downstream on-chip)
                    nc.vector.tensor_scalar(zo[:, sl], hin[:, sl], rstd, nmr,
                                            Alu.mult, Alu.add)
                    nc.sync.dma_start(out_d[st_ * P:(st_ + 1) * P, sl],
                                      zo[:, sl])

        # ---- pipelined emission: transposes hide behind the next out-proj;
        # FFN2(half 0) interposes before the last transpose so z7's LN chain
        # has a full matmul block to complete under.
        hins = {}
        hins[0] = outproj(0)
        hins[1] = outproj(1)
        ln1(0, hins.pop(0))
        hins[2] = outproj(2)
        ln1(1, hins.pop(1))
        transpose_z(0)
        hins[3] = outproj(3)
        ln1(2, hins.pop(2))
        transpose_z(1)
        hins[4] = outproj(4)
        ln1(3, hins.pop(3))
        transpose_z(2)
        hins[5] = outproj(5)
        ln1(4, hins.pop(4))
        transpose_z(3)
        ffn1(0)
        hins[6] = outproj(6)
        ln1(5, hins.pop(5))
        transpose_z(4)
        hins[7] = outproj(7)
        ln1(6, hins.pop(6))
        transpose_z(5)
        ln1(7, hins.pop(7))
        ffn2(0)
        transpose_z(6)
        transpose_z(7)
        ffn1(1)
        ffn2(1)

        pout.release()
        pffn.release()
        pxh.release()
        ph1t.release()
        ph1.release()
        pav.release()
        pln.release()
        pwo.release()
        pw1.release()
        pw2.release()
        sp.release()
        pps.release()
        pp.release()
        cp.release()

    nc.finalize()
    return nc


def _host_prep(inputs):
    import ml_dtypes
    f16 = np.float16
    f32 = np.float32
    f8 = ml_dtypes.float8_e4m3fn
    X = np.asarray(inputs["X"], f32)
    I = np.asarray(inputs["intensity"], f32)

    W1 = np.asarray(inputs["W1"], np.float64)
    g1 = np.asarray(inputs["g1"], np.float64)
    be1 = np.asarray(inputs["be1"], np.float64)
    W1p = (W1 * g1[:, None]).astype(np.float32)
    b1p = (np.asarray(inputs["b1"], np.float64) + be1 @ W1).astype(np.float32)
    # fp8 weights with power-of-2 prescales; see the module docstring
    w1t4 = np.ascontiguousarray(
        (256.0 * W1p).astype(f8).reshape(NK, P, NF, P).transpose(2, 1, 0, 3)
    )
    shared = {
        "wq8": np.asarray(inputs["Wq"], np.float32).astype(f8),
        "wk8": np.asarray(inputs["Wk"], np.float32).astype(f8),
        "wv": np.asarray(inputs["Wv"], f16),
        "wo": np.asarray(inputs["Wo"], f16),
        "w1t4": w1t4,
        "w2": (64.0 * np.asarray(inputs["W2"], np.float32)).astype(f8),
        "bq_p": np.ascontiguousarray(np.asarray(inputs["bq"], f32).reshape(NK, P).T),
        "bk_p": np.ascontiguousarray(np.asarray(inputs["bk"], f32).reshape(NK, P).T),
        "bvr": np.ascontiguousarray(
            np.broadcast_to(np.asarray(inputs["bv"], f16)[None, :], (P, D))
        ),
        "b1_p": np.ascontiguousarray((8.0 * b1p).reshape(NF, P).T),
        "b2r": np.ascontiguousarray(np.broadcast_to(
            (512.0 * (np.asarray(inputs["b2"], np.float64)
                      + np.asarray(inputs["be1"], np.float64))
             ).astype(f16)[None, :],
            (P, D))),
        "g1r": np.ascontiguousarray(np.broadcast_to(
            (512.0 * np.asarray(inputs["g1"], np.float64)).astype(f16)[None, :],
            (P, D))),
    }

    in_maps = []
    for c in range(8):
        b, h = divmod(c, 2)
        own = slice(h * SQ, (h + 1) * SQ)
        oth = slice((1 - h) * SQ, (2 - h) * SQ)
        # sk order: own query rows first, then the other half, so q^T is a
        # contiguous slice of X^T. intensity columns follow the same order.
        xb = np.concatenate([X[b, own], X[b, oth]], axis=0)
        Ih = I[b, own]
        intT = np.concatenate([Ih[:, own], Ih[:, oth]], axis=1).T
        m = dict(shared)
        xbT = np.ascontiguousarray(xb.T.astype(f16))
        m["xbT"] = xbT
        m["xbT8"] = xbT.astype(f8)
        m["xh16"] = (X[b, own] + np.asarray(inputs["bo"], f32)[None, :]).astype(f16)
        m["intT"] = np.ascontiguousarray(intT.astype(f16))
        in_maps.append(m)
    return in_maps


def kernel(**inputs) -> np.ndarray:
    global _PROG
    if _PROG is None:
        _PROG = _build()
    from concourse.bass_utils import run_bass_kernel_spmd

    in_maps = _host_prep(inputs)
    res = run_bass_kernel_spmd(_PROG, in_maps, list(range(8)))
    out = np.empty((B, S, D), np.float32)
    for c, r in enumerate(res.results):
        b, h = divmod(c, 2)
        out[b, h * SQ:(h + 1) * SQ] = r["out"]
    g2 = np.asarray(inputs["g2"], np.float32)
    be2 = np.asarray(inputs["be2"], np.float32)
    return out * g2 + be2
